# revision 1
# baseline (speedup 1.0000x reference)
"""Trainium2 Bass kernel for the CoupledTauModel (gnn_message_passing).

Strategy
--------
All math runs in a transposed "nodes-on-partitions" layout: the state lives as
usT/ufT [N, B] sharded so that core c owns output rows [c*LOC, (c+1)*LOC).

Host-side prep folds the Euler update into a single streamed matrix per state:
    A_s = I - dt*c_s*Ls.T          (shape [N, N], core c gets columns c-slice)
so one PSUM accumulation computes  us + dt*(-c_s * us@Ls.T)  directly, and the
low-rank coupling term  dt*l_s * uf@Ms.T  is added into the same PSUM group via
two tiny matmuls (rank 5).  Epilogue is a single Relu activation per tile.

Per step each core:
  1. p_sT = Ms_B.T @ ufT, p_fT = Mf_B.T @ usT        (tiny, rank-5)
  2. for its 1024 output rows: psum = A_s_slice.T @ usT + MsA_scaled.T @ p_sT
  3. new local state = Relu(psum)
  4. AllGather the [2*LOC, B] local state update across the 8 cores.

The 256MB Ls/Lf-derived matrices are streamed from HBM every step (they cannot
fit in SBUF) in 512KB contiguous slabs -> the kernel is HBM-bandwidth-bound,
which is the target regime.

The gate MLP (sigmoid split of x0) and decoder MLP run on-device in the same
transposed layout; W1/D1 are replicated, W2/D2 column-sharded.
"""

import math
from contextlib import ExitStack

import numpy as np

import concourse.bass as bass
import concourse.mybir as mybir
import concourse.tile as tile
from concourse import bacc
from concourse.bass_utils import run_bass_kernel_spmd

AF = mybir.ActivationFunctionType
FP32 = mybir.dt.float32

# Full-size problem config (hardcoded; the harness always uses this shape).
N_FULL = 8192
B_FULL = 8
H_FULL = 128
R_FULL = 5
NCORES = 8
NSTEPS_FULL = 10
DT = 0.1


def build_program(N=N_FULL, B=B_FULL, H=H_FULL, R=R_FULL, ncores=NCORES,
                  nsteps=NSTEPS_FULL, slab_bufs=16, pin=0,
                  stream_dma=True, do_mm=True, do_ag=True):
    """Build the SPMD Bass program (identical on every core; per-core data
    comes from each core's input map)."""
    LOC = N // ncores          # output rows owned per core
    KT = N // 128              # contraction tiles
    MT = LOC // 128            # output tiles per core
    assert N % (128 * ncores) == 0 and H == 128

    nc = bacc.Bacc("TRN2", target_bir_lowering=False, debug=False,
                   num_devices=ncores)
    dt = FP32

    x0T = nc.dram_tensor("x0T", [N, B], dt, kind="ExternalInput")
    x0Tl = nc.dram_tensor("x0Tl", [LOC, B], dt, kind="ExternalInput")
    As_d = nc.dram_tensor("As", [N, LOC], dt, kind="ExternalInput")
    Af_d = nc.dram_tensor("Af", [N, LOC], dt, kind="ExternalInput")
    MsB_d = nc.dram_tensor("MsB", [N, R], dt, kind="ExternalInput")
    MfB_d = nc.dram_tensor("MfB", [N, R], dt, kind="ExternalInput")
    MsA_d = nc.dram_tensor("MsA", [R, LOC], dt, kind="ExternalInput")
    MfA_d = nc.dram_tensor("MfA", [R, LOC], dt, kind="ExternalInput")
    W1_d = nc.dram_tensor("W1", [N, H], dt, kind="ExternalInput")
    b1_d = nc.dram_tensor("b1", [H, 1], dt, kind="ExternalInput")
    W2_d = nc.dram_tensor("W2", [H, LOC], dt, kind="ExternalInput")
    b2T_d = nc.dram_tensor("b2T", [128, MT], dt, kind="ExternalInput")
    D1_d = nc.dram_tensor("D1", [N, H], dt, kind="ExternalInput")
    db1_d = nc.dram_tensor("db1", [H, 1], dt, kind="ExternalInput")
    D2_d = nc.dram_tensor("D2", [H, LOC], dt, kind="ExternalInput")
    db2T_d = nc.dram_tensor("db2T", [128, MT], dt, kind="ExternalInput")

    x1_o = nc.dram_tensor("x1_o", [LOC, B], dt, kind="ExternalOutput")
    us_o = nc.dram_tensor("us_o", [LOC, B], dt, kind="ExternalOutput")
    uf_o = nc.dram_tensor("uf_o", [LOC, B], dt, kind="ExternalOutput")

    RG = [list(range(ncores))]

    with ExitStack() as ctx:
        tc = ctx.enter_context(tile.TileContext(nc))
        const = ctx.enter_context(tc.tile_pool(name="const", bufs=1))
        statep = ctx.enter_context(tc.tile_pool(name="state", bufs=2))
        slabp = ctx.enter_context(tc.tile_pool(name="slab", bufs=slab_bufs))
        mlpp = ctx.enter_context(tc.tile_pool(name="mlp", bufs=4))
        smallp = ctx.enter_context(tc.tile_pool(name="small", bufs=4))
        newp = ctx.enter_context(tc.tile_pool(name="new", bufs=2))
        psum = ctx.enter_context(tc.tile_pool(name="psum", bufs=8, space="PSUM"))
        dram = ctx.enter_context(tc.tile_pool(name="dram", bufs=2, space="DRAM"))

        # ---- resident constants ----
        x0T_sb = const.tile([128, KT, B], dt, tag="x0T")
        nc.sync.dma_start(x0T_sb[:], x0T[:].rearrange("(k p) b -> p k b", p=128))
        x0Tl_sb = const.tile([128, MT, B], dt, tag="x0Tl")
        nc.sync.dma_start(x0Tl_sb[:], x0Tl[:].rearrange("(m p) b -> p m b", p=128))
        MsB_sb = const.tile([128, KT, R], dt, tag="MsB")
        nc.sync.dma_start(MsB_sb[:], MsB_d[:].rearrange("(k p) r -> p k r", p=128))
        MfB_sb = const.tile([128, KT, R], dt, tag="MfB")
        nc.sync.dma_start(MfB_sb[:], MfB_d[:].rearrange("(k p) r -> p k r", p=128))
        MsA_sb = const.tile([R, LOC], dt, tag="MsA")
        nc.sync.dma_start(MsA_sb[:], MsA_d[:])
        MfA_sb = const.tile([R, LOC], dt, tag="MfA")
        nc.sync.dma_start(MfA_sb[:], MfA_d[:])
        b1_sb = const.tile([H, 1], dt, tag="b1")
        nc.sync.dma_start(b1_sb[:], b1_d[:])
        db1_sb = const.tile([H, 1], dt, tag="db1")
        nc.sync.dma_start(db1_sb[:], db1_d[:])
        b2_sb = const.tile([128, MT], dt, tag="b2")
        nc.sync.dma_start(b2_sb[:], b2T_d[:])
        db2_sb = const.tile([128, MT], dt, tag="db2")
        nc.sync.dma_start(db2_sb[:], db2T_d[:])

        # ---- pinned A slabs: loaded once, reused every step ----
        pinned = [[], []]
        if pin:
            pinp = ctx.enter_context(tc.tile_pool(name="pin", bufs=1))
            for ph, A_d in enumerate([As_d, Af_d]):
                for i in range(pin):
                    ptile = pinp.tile([128, LOC], dt, tag=f"pin{ph}_{i}",
                                      name=f"pin{ph}_{i}")
                    nc.sync.dma_start(ptile[:], A_d[i * 128:(i + 1) * 128, :])
                    pinned[ph].append(ptile)

        # ---- gate MLP:  gate = sigmoid(relu(x0@W1+b1)@W2+b2) ----
        hpsum = psum.tile([H, B], dt, tag="mm")
        for k in range(KT):
            w1t = mlpp.tile([128, H], dt, tag="w1")
            nc.sync.dma_start(w1t[:], W1_d[k * 128:(k + 1) * 128, :])
            nc.tensor.matmul(hpsum[:], w1t[:], x0T_sb[:, k, :],
                             start=(k == 0), stop=(k == KT - 1))
        hT = smallp.tile([H, B], dt, tag="hT")
        nc.scalar.activation(hT[:], hpsum[:], AF.Relu, bias=b1_sb[:, 0:1])

        w2_sb = mlpp.tile([H, LOC], dt, tag="w2")
        nc.sync.dma_start(w2_sb[:], W2_d[:])
        news0 = newp.tile([128, MT, B], dt, tag="new")
        newf0 = newp.tile([128, MT, B], dt, tag="newf")
        for m in range(MT):
            gpsum = psum.tile([128, B], dt, tag="mm")
            nc.tensor.matmul(gpsum[:], w2_sb[:, m * 128:(m + 1) * 128], hT[:],
                             start=True, stop=True)
            gate = smallp.tile([128, B], dt, tag="gate")
            nc.scalar.activation(gate[:], gpsum[:], AF.Sigmoid, bias=b2_sb[:, m:m + 1])
            nc.vector.tensor_mul(news0[:, m, :], gate[:], x0Tl_sb[:, m, :])
            nc.vector.tensor_sub(newf0[:, m, :], x0Tl_sb[:, m, :], news0[:, m, :])

        def exchange(news, newf):
            """AllGather local [2*LOC, B] update -> fresh full-state tiles."""
            agin = dram.tile([2 * LOC, B], dt, tag="agin")
            nc.sync.dma_start(
                agin[0:LOC, :].rearrange("(m p) b -> p m b", p=128), news[:])
            nc.sync.dma_start(
                agin[LOC:2 * LOC, :].rearrange("(m p) b -> p m b", p=128), newf[:])
            agout = dram.tile([ncores * 2 * LOC, B], dt, tag="agout")
            if do_ag:
                nc.gpsimd.collective_compute(
                    "AllGather", mybir.AluOpType.bypass, replica_groups=RG,
                    ins=[agin[:]], outs=[agout[:]])
            else:
                nc.sync.dma_start(agout[0:2 * LOC, :], agin[:])
            # Gather the rank-interleaved AG output into contiguous [N, B]
            # DRAM buffers (one DMA each), then load SBUF state with one DMA
            # per tensor -- keeps the semaphore fan-in per consumer tiny.
            us_lin = dram.tile([N, B], dt, tag="uslin")
            uf_lin = dram.tile([N, B], dt, tag="uflin")
            agv = agout[:].rearrange("(r t q) b -> r t q b", t=2, q=LOC)
            nc.gpsimd.dma_start(
                us_lin[:].rearrange("(r q) b -> r q b", q=LOC), agv[:, 0])
            nc.gpsimd.dma_start(
                uf_lin[:].rearrange("(r q) b -> r q b", q=LOC), agv[:, 1])
            us_nx = statep.tile([128, KT, B], dt, tag="us")
            uf_nx = statep.tile([128, KT, B], dt, tag="uf")
            nc.gpsimd.dma_start(
                us_nx[:], us_lin[:].rearrange("(k p) b -> p k b", p=128))
            nc.gpsimd.dma_start(
                uf_nx[:], uf_lin[:].rearrange("(k p) b -> p k b", p=128))
            return agin, us_nx, uf_nx

        _, us_cur, uf_cur = exchange(news0, newf0)

        # ---- 10 Euler steps ----
        last_agin = None
        for t in range(nsteps):
            # rank-R projections p_sT = MsB.T @ ufT, p_fT = MfB.T @ usT
            pps = psum.tile([R, B], dt, tag="mm")
            for k in range(KT):
                nc.tensor.matmul(pps[:], MsB_sb[:, k, :], uf_cur[:, k, :],
                                 start=(k == 0), stop=(k == KT - 1))
            ppf = psum.tile([R, B], dt, tag="mm")
            for k in range(KT):
                nc.tensor.matmul(ppf[:], MfB_sb[:, k, :], us_cur[:, k, :],
                                 start=(k == 0), stop=(k == KT - 1))
            ps_sb = smallp.tile([R, B], dt, tag="p")
            nc.vector.tensor_copy(ps_sb[:], pps[:])
            pf_sb = smallp.tile([R, B], dt, tag="p")
            nc.vector.tensor_copy(pf_sb[:], ppf[:])

            news = newp.tile([128, MT, B], dt, tag="new")
            newf = newp.tile([128, MT, B], dt, tag="newf")
            for phase in range(2):
                A_d = As_d if phase == 0 else Af_d
                st = us_cur if phase == 0 else uf_cur
                MA = MsA_sb if phase == 0 else MfA_sb
                pp = ps_sb if phase == 0 else pf_sb
                dst = news if phase == 0 else newf
                mps = [psum.tile([128, B], dt, tag="mm", name=f"mm_{t}_{phase}_{m}")
                       for m in range(MT)]
                resident = None
                if not stream_dma:
                    resident = slabp.tile([128, LOC], dt, tag="slab",
                                          name=f"res_{t}_{phase}")
                    nc.sync.dma_start(resident[:], A_d[0:128, :])
                for k in range(KT):
                    if k < len(pinned[phase]):
                        slab = pinned[phase][k]
                    elif stream_dma:
                        slab = slabp.tile([128, LOC], dt, tag="slab",
                                          name=f"slab_{t}_{phase}_{k}")
                        nc.sync.dma_start(slab[:], A_d[k * 128:(k + 1) * 128, :])
                    else:
                        slab = resident
                    if do_mm:
                        for m in range(MT):
                            nc.tensor.matmul(mps[m][:],
                                             slab[:, m * 128:(m + 1) * 128],
                                             st[:, k, :], start=(k == 0),
                                             stop=False)
                for m in range(MT):
                    nc.tensor.matmul(mps[m][:], MA[:, m * 128:(m + 1) * 128], pp[:],
                                     start=False, stop=True)
                    nc.scalar.activation(dst[:, m, :], mps[m][:], AF.Relu)

            last_agin, us_cur, uf_cur = exchange(news, newf)

        # final local state -> outputs (DRAM->DRAM copy out of the AG input)
        nc.sync.dma_start(us_o[:], last_agin[0:LOC, :])
        nc.sync.dma_start(uf_o[:], last_agin[LOC:2 * LOC, :])

        # ---- decoder:  x1 = softplus(relu((us+uf)@D1+db1)@D2+db2) ----
        lat = statep.tile([128, KT, B], dt, tag="lat")
        nc.vector.tensor_add(lat[:], us_cur[:], uf_cur[:])
        hdp = psum.tile([H, B], dt, tag="mm")
        for k in range(KT):
            d1t = mlpp.tile([128, H], dt, tag="w1")
            nc.sync.dma_start(d1t[:], D1_d[k * 128:(k + 1) * 128, :])
            nc.tensor.matmul(hdp[:], d1t[:], lat[:, k, :],
                             start=(k == 0), stop=(k == KT - 1))
        hdT = smallp.tile([H, B], dt, tag="hT")
        nc.scalar.activation(hdT[:], hdp[:], AF.Relu, bias=db1_sb[:, 0:1])
        d2_sb = mlpp.tile([H, LOC], dt, tag="w2")
        nc.sync.dma_start(d2_sb[:], D2_d[:])
        x1sb = newp.tile([128, MT, B], dt, tag="new")
        for m in range(MT):
            xp = psum.tile([128, B], dt, tag="mm")
            nc.tensor.matmul(xp[:], d2_sb[:, m * 128:(m + 1) * 128], hdT[:],
                             start=True, stop=True)
            # stable softplus(x+db2) = relu(x+db2) + ln(1+exp(-|x+db2|))
            xa = smallp.tile([128, B], dt, tag="xa")
            nc.scalar.activation(xa[:], xp[:], AF.Abs, bias=db2_sb[:, m:m + 1])
            nc.scalar.activation(xa[:], xa[:], AF.Exp, scale=-1.0)
            nc.scalar.activation(xa[:], xa[:], AF.Ln, bias=1.0)
            xr = smallp.tile([128, B], dt, tag="xr")
            nc.scalar.activation(xr[:], xp[:], AF.Relu, bias=db2_sb[:, m:m + 1])
            nc.vector.tensor_add(x1sb[:, m, :], xr[:], xa[:])
        nc.sync.dma_start(x1_o[:].rearrange("(m p) b -> p m b", p=128), x1sb[:])

    nc.compile()
    return nc


def make_in_maps(inputs, N=N_FULL, B=B_FULL, H=H_FULL, R=R_FULL,
                 ncores=NCORES):
    """Host-side prep: fold scalars/identity into the streamed matrices and
    shard across cores.  Returns a list of per-core input dicts."""
    LOC = N // ncores
    MT = LOC // 128
    f32 = np.float32

    def softplus(x):
        return np.log1p(np.exp(np.float64(x)))

    a_s = f32(DT * (softplus(inputs["raw_cs"]) + 1e-4))
    a_f = f32(DT * (softplus(inputs["raw_cf"]) + 1e-4))
    b_s = f32(DT * (softplus(inputs["raw_lambda_s"]) + 1e-4))
    b_f = f32(DT * (softplus(inputs["raw_lambda_f"]) + 1e-4))

    Ls = np.asarray(inputs["Ls"], f32)
    Lf = np.asarray(inputs["Lf"], f32)
    x0 = np.asarray(inputs["x0"], f32)
    x0T = np.ascontiguousarray(x0.T)

    com = {
        "x0T": x0T,
        "MsB": np.ascontiguousarray(np.asarray(inputs["Ms_B"], f32)),
        "MfB": np.ascontiguousarray(np.asarray(inputs["Mf_B"], f32)),
        "W1": np.ascontiguousarray(np.asarray(inputs["W1"], f32)),
        "b1": np.ascontiguousarray(np.asarray(inputs["b1"], f32).reshape(H, 1)),
        "D1": np.ascontiguousarray(np.asarray(inputs["D1"], f32)),
        "db1": np.ascontiguousarray(np.asarray(inputs["db1"], f32).reshape(H, 1)),
    }
    W2 = np.asarray(inputs["W2"], f32)
    D2 = np.asarray(inputs["D2"], f32)
    b2 = np.asarray(inputs["b2"], f32)
    db2 = np.asarray(inputs["db2"], f32)
    MsA = np.asarray(inputs["Ms_A"], f32)
    MfA = np.asarray(inputs["Mf_A"], f32)

    in_maps = []
    diag = np.arange(LOC)
    for c in range(ncores):
        r0, r1 = c * LOC, (c + 1) * LOC
        As_c = (-a_s) * Ls[r0:r1, :].T
        As_c[r0 + diag, diag] += f32(1.0)
        Af_c = (-a_f) * Lf[r0:r1, :].T
        Af_c[r0 + diag, diag] += f32(1.0)
        m = dict(com)
        m.update({
            "x0Tl": np.ascontiguousarray(x0T[r0:r1]),
            "As": np.ascontiguousarray(As_c),
            "Af": np.ascontiguousarray(Af_c),
            "MsA": np.ascontiguousarray(b_s * MsA[r0:r1].T),
            "MfA": np.ascontiguousarray(b_f * MfA[r0:r1].T),
            "W2": np.ascontiguousarray(W2[:, r0:r1]),
            "b2T": np.ascontiguousarray(b2[r0:r1].reshape(MT, 128).T),
            "D2": np.ascontiguousarray(D2[:, r0:r1]),
            "db2T": np.ascontiguousarray(db2[r0:r1].reshape(MT, 128).T),
        })
        in_maps.append(m)
    return in_maps, (a_s, a_f, b_s, b_f)


_PROGRAM_CACHE = {}


def kernel(**inputs):
    """Full-input / full-output entry point for the harness."""
    in_maps, _scal = make_in_maps(inputs)
    key = "full"
    if key not in _PROGRAM_CACHE:
        _PROGRAM_CACHE[key] = build_program()
    nc = _PROGRAM_CACHE[key]

    res = run_bass_kernel_spmd(nc, in_maps, core_ids=list(range(NCORES)))

    x1T = np.concatenate([res.results[c]["x1_o"] for c in range(NCORES)], axis=0)
    usT = np.concatenate([res.results[c]["us_o"] for c in range(NCORES)], axis=0)
    ufT = np.concatenate([res.results[c]["uf_o"] for c in range(NCORES)], axis=0)
    x1 = np.ascontiguousarray(x1T.T).astype(np.float32)
    us = np.ascontiguousarray(usT.T).astype(np.float32)
    uf = np.ascontiguousarray(ufT.T).astype(np.float32)
    return (x1, us, uf)



# revision 2
# speedup vs baseline: 3.4635x; 3.4635x over previous
"""Trainium2 Bass kernel for the CoupledTauModel (gnn_message_passing).

Strategy (v2 — flipped matmul orientation, bf16)
------------------------------------------------
The Euler recurrence  us' = relu(us@(I-a_s*Ls).T + b_s*(uf@Ms_B)@Ms_A.T)
is computed with the *state* as the 128x8 stationary operand and the folded
matrix A = I - a*L streamed from HBM as the bf16 *moving* operand in 512-col
chunks, accumulating PSUM tiles of shape [B=8, 512].  This replaces the v1
orientation (A stationary, 8-wide moving state) which paid a 128x128
LDWEIGHTS per 8 moving columns and ran fp32 (2 HW passes per matmul).

Sharding: core c owns output columns [c*LOC, (c+1)*LOC) of A (column/output
-node sharding).  After each phase the new local state slice [8, LOC] is
transposed on the PE (8x [8,128] -> [128,8]) back into node-major layout and
AllGathered per state, so each collective hides under the other phase's
compute.

Everything on the matmul path is bf16 (verified 8.3e-3 worst rel err vs the
fp32 reference, under the 2e-2 gate); PSUM accumulation stays fp32.
"""

import math
from contextlib import ExitStack

import numpy as np
import ml_dtypes

import concourse.bass as bass
import concourse.mybir as mybir
import concourse.tile as tile
from concourse import bacc
from concourse.bass_utils import run_bass_kernel_spmd

AF = mybir.ActivationFunctionType
FP32 = mybir.dt.float32
BF16 = mybir.dt.bfloat16
NP_BF16 = ml_dtypes.bfloat16

# Full-size problem config (hardcoded; the harness always uses this shape).
N_FULL = 8192
B_FULL = 8
H_FULL = 128
R_FULL = 5
NCORES = 8
NSTEPS_FULL = 10
DT = 0.1


def build_program(N=N_FULL, B=B_FULL, H=H_FULL, R=R_FULL, ncores=NCORES,
                  nsteps=NSTEPS_FULL, kchunk=8, slab_bufs=4, pin_chunks=0,
                  do_ag=True):
    """Build the SPMD Bass program (identical on every core; per-core data
    comes from each core's input map)."""
    LOC = N // ncores          # output columns owned per core
    KT = N // 128              # contraction k-tiles
    MT = LOC // 128            # 128-row tiles per local slice
    NJ = LOC // 512            # psum output chunks per phase
    NCH = KT // kchunk         # streamed slab chunks per phase
    assert N % (128 * ncores) == 0 and H == 128 and LOC % 512 == 0
    assert KT % kchunk == 0 and pin_chunks <= NCH

    nc = bacc.Bacc("TRN2", target_bir_lowering=False, debug=False,
                   num_devices=ncores)

    # ---- DRAM inputs (bf16 unless noted) ----
    As_d = nc.dram_tensor("As", [128, KT, LOC], BF16, kind="ExternalInput")
    Af_d = nc.dram_tensor("Af", [128, KT, LOC], BF16, kind="ExternalInput")
    x0T_d = nc.dram_tensor("x0T", [N, B], BF16, kind="ExternalInput")
    x0lb_d = nc.dram_tensor("x0lb", [B, LOC], FP32, kind="ExternalInput")
    MsB_d = nc.dram_tensor("MsB", [N, R], BF16, kind="ExternalInput")
    MfB_d = nc.dram_tensor("MfB", [N, R], BF16, kind="ExternalInput")
    MsAT_d = nc.dram_tensor("MsAT", [R, LOC], BF16, kind="ExternalInput")
    MfAT_d = nc.dram_tensor("MfAT", [R, LOC], BF16, kind="ExternalInput")
    W1_d = nc.dram_tensor("W1r", [128, KT, H], BF16, kind="ExternalInput")
    W2_d = nc.dram_tensor("W2", [H, LOC], BF16, kind="ExternalInput")
    D1_d = nc.dram_tensor("D1r", [128, KT, H], BF16, kind="ExternalInput")
    D2_d = nc.dram_tensor("D2", [H, LOC], BF16, kind="ExternalInput")
    b1r_d = nc.dram_tensor("b1r", [1, H], BF16, kind="ExternalInput")
    b2r_d = nc.dram_tensor("b2r", [1, LOC], BF16, kind="ExternalInput")
    db1r_d = nc.dram_tensor("db1r", [1, H], BF16, kind="ExternalInput")
    db2r_d = nc.dram_tensor("db2r", [1, LOC], BF16, kind="ExternalInput")
    ones_d = nc.dram_tensor("ones8", [1, B], BF16, kind="ExternalInput")
    eye_d = nc.dram_tensor("eye8", [B, B], BF16, kind="ExternalInput")

    x1_o = nc.dram_tensor("x1_o", [B, LOC], FP32, kind="ExternalOutput")
    us_o = nc.dram_tensor("us_o", [LOC, B], BF16, kind="ExternalOutput")
    uf_o = nc.dram_tensor("uf_o", [LOC, B], BF16, kind="ExternalOutput")

    RG = [list(range(ncores))]

    with ExitStack() as ctx:
        tc = ctx.enter_context(tile.TileContext(nc))
        const = ctx.enter_context(tc.tile_pool(name="const", bufs=1))
        statep = ctx.enter_context(tc.tile_pool(name="state", bufs=3))
        slabp = ctx.enter_context(tc.tile_pool(name="slab", bufs=slab_bufs))
        mlpp = ctx.enter_context(tc.tile_pool(name="mlp", bufs=1))
        smallp = ctx.enter_context(tc.tile_pool(name="small", bufs=2))
        newp = ctx.enter_context(tc.tile_pool(name="new", bufs=2))
        psum = ctx.enter_context(tc.tile_pool(name="psum", bufs=1, space="PSUM"))
        dram = ctx.enter_context(tc.tile_pool(name="dram", bufs=2, space="DRAM"))

        # ---- resident constants ----
        x0T_sb = const.tile([128, KT, B], BF16, tag="x0T")
        nc.sync.dma_start(x0T_sb[:], x0T_d[:].rearrange("(k p) b -> p k b", p=128))
        x0lb_sb = const.tile([B, LOC], FP32, tag="x0lb")
        nc.sync.dma_start(x0lb_sb[:], x0lb_d[:])
        MsB_sb = const.tile([128, KT, R], BF16, tag="MsB")
        nc.sync.dma_start(MsB_sb[:], MsB_d[:].rearrange("(k p) r -> p k r", p=128))
        MfB_sb = const.tile([128, KT, R], BF16, tag="MfB")
        nc.sync.dma_start(MfB_sb[:], MfB_d[:].rearrange("(k p) r -> p k r", p=128))
        MsAT_sb = const.tile([R, LOC], BF16, tag="MsAT")
        nc.sync.dma_start(MsAT_sb[:], MsAT_d[:])
        MfAT_sb = const.tile([R, LOC], BF16, tag="MfAT")
        nc.sync.dma_start(MfAT_sb[:], MfAT_d[:])
        W2_sb = const.tile([H, LOC], BF16, tag="W2")
        nc.sync.dma_start(W2_sb[:], W2_d[:])
        D2_sb = const.tile([H, LOC], BF16, tag="D2")
        nc.sync.dma_start(D2_sb[:], D2_d[:])
        b1r_sb = const.tile([1, H], BF16, tag="b1r")
        nc.sync.dma_start(b1r_sb[:], b1r_d[:])
        b2r_sb = const.tile([1, LOC], BF16, tag="b2r")
        nc.sync.dma_start(b2r_sb[:], b2r_d[:])
        db1r_sb = const.tile([1, H], BF16, tag="db1r")
        nc.sync.dma_start(db1r_sb[:], db1r_d[:])
        db2r_sb = const.tile([1, LOC], BF16, tag="db2r")
        nc.sync.dma_start(db2r_sb[:], db2r_d[:])
        ones_sb = const.tile([1, B], BF16, tag="ones")
        nc.sync.dma_start(ones_sb[:], ones_d[:])
        eye_sb = const.tile([B, B], BF16, tag="eye")
        nc.sync.dma_start(eye_sb[:], eye_d[:])

        # ---- pinned A chunks: loaded once, reused every step ----
        pinned = [[], []]
        if pin_chunks:
            pinp = ctx.enter_context(tc.tile_pool(name="pin", bufs=1))
            for ph, A_d in enumerate([As_d, Af_d]):
                for c in range(pin_chunks):
                    pt_ = pinp.tile([128, kchunk, LOC], BF16,
                                    tag=f"pin{ph}_{c}", name=f"pin{ph}_{c}")
                    nc.sync.dma_start(
                        pt_[:], A_d[:, c * kchunk:(c + 1) * kchunk, :])
                    pinned[ph].append(pt_)

        def bT_to_nT(src_bT, nm):
            """[B, LOC] bf16 SBUF -> node-major [128, MT, B] bf16 tile via PE
            transposes."""
            news = newp.tile([128, MT, B], BF16, tag="news", name=f"news_{nm}")
            for m in range(MT):
                pt = psum.tile([128, B], BF16, tag="pt", bufs=2,
                               name=f"pt_{nm}_{m}")
                nc.tensor.transpose(pt[:], src_bT[:, m * 128:(m + 1) * 128],
                                    eye_sb[:])
                nc.vector.tensor_copy(news[:, m, :], pt[:])
            return news

        def exchange(news, nm):
            """AllGather one state's local [LOC, B] update -> full [128,KT,B]
            stationary tile."""
            agin = dram.tile([LOC, B], BF16, tag=f"agin_{nm[-1]}",
                             name=f"agin_{nm}")
            nc.sync.dma_start(
                agin[:].rearrange("(m p) b -> p m b", p=128), news[:])
            agout = dram.tile([N, B], BF16, tag=f"agout_{nm[-1]}",
                              name=f"agout_{nm}")
            if do_ag:
                nc.gpsimd.collective_compute(
                    "AllGather", mybir.AluOpType.bypass, replica_groups=RG,
                    ins=[agin[:]], outs=[agout[:]])
            else:
                nc.sync.dma_start(agout[0:LOC, :], agin[:])
            st = statep.tile([128, KT, B], BF16, tag=f"st_{nm[-1]}",
                             name=f"st_{nm}")
            nc.sync.dma_start(
                st[:], agout[:].rearrange("(k p) b -> p k b", p=128))
            return agin, st

        # ---- gate MLP:  gate = sigmoid(relu(x0@W1+b1)@W2+b2) ----
        W1_sb = mlpp.tile([128, KT, H], BF16, tag="w1r")
        nc.sync.dma_start(W1_sb[:], W1_d[:])
        hp = psum.tile([B, H], FP32, tag="q")
        for k in range(KT):
            nc.tensor.matmul(hp[:], x0T_sb[:, k, :], W1_sb[:, k, :],
                             start=(k == 0), stop=False)
        nc.tensor.matmul(hp[:], ones_sb[:], b1r_sb[:], start=False, stop=True)
        h_sb = smallp.tile([B, H], BF16, tag="h")
        nc.scalar.activation(h_sb[:], hp[:], AF.Relu)
        htp = psum.tile([128, B], BF16, tag="pt", bufs=2)
        nc.tensor.transpose(htp[:], h_sb[:], eye_sb[:])
        hT_sb = smallp.tile([H, B], BF16, tag="hT")
        nc.vector.tensor_copy(hT_sb[:], htp[:])

        gate_sb = smallp.tile([B, LOC], FP32, tag="gate", bufs=1)
        for j in range(NJ):
            gp = psum.tile([B, 512], FP32, tag=f"out{j}", bufs=2,
                           name=f"gp_{j}")
            nc.tensor.matmul(gp[:], hT_sb[:], W2_sb[:, j * 512:(j + 1) * 512],
                             start=True, stop=False)
            nc.tensor.matmul(gp[:], ones_sb[:],
                             b2r_sb[:, j * 512:(j + 1) * 512],
                             start=False, stop=True)
            nc.scalar.activation(gate_sb[:, j * 512:(j + 1) * 512], gp[:],
                                 AF.Sigmoid)
        us0_b = smallp.tile([B, LOC], FP32, tag="us0b", bufs=1)
        nc.vector.tensor_mul(us0_b[:], gate_sb[:], x0lb_sb[:])
        uf0_b = smallp.tile([B, LOC], FP32, tag="uf0b", bufs=1)
        nc.vector.tensor_sub(uf0_b[:], x0lb_sb[:], us0_b[:])
        us0_bT = smallp.tile([B, LOC], BF16, tag="new_bT", bufs=2)
        nc.vector.tensor_copy(us0_bT[:], us0_b[:])
        uf0_bT = smallp.tile([B, LOC], BF16, tag="new_bT", bufs=2)
        nc.vector.tensor_copy(uf0_bT[:], uf0_b[:])

        news0 = bT_to_nT(us0_bT, "s0")
        newf0 = bT_to_nT(uf0_bT, "f0")
        _, us_cur = exchange(news0, "us0")
        _, uf_cur = exchange(newf0, "uf0")

        # ---- Euler steps ----
        last_agin = [None, None]
        for t in range(nsteps):
            new_st = [None, None]
            for ph in range(2):
                A_d = As_d if ph == 0 else Af_d
                stA = us_cur if ph == 0 else uf_cur      # stationary state
                stQ = uf_cur if ph == 0 else us_cur      # rank-term state
                qB = MsB_sb if ph == 0 else MfB_sb
                MAT = MsAT_sb if ph == 0 else MfAT_sb
                nm = f"{t}_{'s' if ph == 0 else 'f'}"

                outs = [psum.tile([B, 512], FP32, tag=f"out{j}", bufs=2,
                                  name=f"out{nm}_{j}") for j in range(NJ)]
                for c in range(NCH):
                    if c < len(pinned[ph]):
                        slab = pinned[ph][c]
                    else:
                        slab = slabp.tile([128, kchunk, LOC], BF16, tag="slab",
                                          name=f"slab_{nm}_{c}")
                        nc.sync.dma_start(
                            slab[:], A_d[:, c * kchunk:(c + 1) * kchunk, :])
                    for kk in range(kchunk):
                        k = c * kchunk + kk
                        for j in range(NJ):
                            nc.tensor.matmul(
                                outs[j][:], stA[:, k, :],
                                slab[:, kk, j * 512:(j + 1) * 512],
                                start=(k == 0), stop=False)
                # rank-R term: q = state @ M_B  (accumulated), then transpose
                qp = psum.tile([B, R], FP32, tag="q", name=f"qp_{nm}")
                for k in range(KT):
                    nc.tensor.matmul(qp[:], stQ[:, k, :], qB[:, k, :],
                                     start=(k == 0), stop=(k == KT - 1))
                q_sb = smallp.tile([B, R], BF16, tag="q", name=f"q_{nm}")
                nc.vector.tensor_copy(q_sb[:], qp[:])
                qtp = psum.tile([R, B], BF16, tag="qt", name=f"qtp_{nm}")
                nc.tensor.transpose(qtp[:], q_sb[:], eye_sb[:])
                qT_sb = smallp.tile([R, B], BF16, tag="qT", name=f"qT_{nm}")
                nc.vector.tensor_copy(qT_sb[:], qtp[:])
                for j in range(NJ):
                    nc.tensor.matmul(outs[j][:], qT_sb[:],
                                     MAT[:, j * 512:(j + 1) * 512],
                                     start=False, stop=True)
                new_bT = smallp.tile([B, LOC], BF16, tag="new_bT", bufs=2,
                                     name=f"newbT_{nm}")
                for j in range(NJ):
                    nc.scalar.activation(new_bT[:, j * 512:(j + 1) * 512],
                                         outs[j][:], AF.Relu)
                news = bT_to_nT(new_bT, nm)
                last_agin[ph], new_st[ph] = exchange(news, nm)
            us_cur, uf_cur = new_st

        # final local state -> outputs (DRAM->DRAM copy out of the AG input)
        nc.sync.dma_start(us_o[:], last_agin[0][:])
        nc.sync.dma_start(uf_o[:], last_agin[1][:])

        # ---- decoder:  x1 = softplus(relu((us+uf)@D1+db1)@D2+db2) ----
        lat = statep.tile([128, KT, B], BF16, tag="lat", bufs=1)
        nc.vector.tensor_add(lat[:], us_cur[:], uf_cur[:])
        D1_sb = mlpp.tile([128, KT, H], BF16, tag="w1r")
        nc.sync.dma_start(D1_sb[:], D1_d[:])
        hdp = psum.tile([B, H], FP32, tag="q")
        for k in range(KT):
            nc.tensor.matmul(hdp[:], lat[:, k, :], D1_sb[:, k, :],
                             start=(k == 0), stop=False)
        nc.tensor.matmul(hdp[:], ones_sb[:], db1r_sb[:], start=False, stop=True)
        hd_sb = smallp.tile([B, H], BF16, tag="h")
        nc.scalar.activation(hd_sb[:], hdp[:], AF.Relu)
        hdtp = psum.tile([128, B], BF16, tag="pt", bufs=2)
        nc.tensor.transpose(hdtp[:], hd_sb[:], eye_sb[:])
        hdT_sb = smallp.tile([H, B], BF16, tag="hT")
        nc.vector.tensor_copy(hdT_sb[:], hdtp[:])

        x1_sb = smallp.tile([B, LOC], FP32, tag="x1", bufs=1)
        for j in range(NJ):
            xp = psum.tile([B, 512], FP32, tag=f"out{j}", bufs=2,
                           name=f"xp_{j}")
            nc.tensor.matmul(xp[:], hdT_sb[:], D2_sb[:, j * 512:(j + 1) * 512],
                             start=True, stop=False)
            nc.tensor.matmul(xp[:], ones_sb[:],
                             db2r_sb[:, j * 512:(j + 1) * 512],
                             start=False, stop=True)
            # stable softplus(y) = relu(y) + ln(1+exp(-|y|))
            sl = slice(j * 512, (j + 1) * 512)
            xa = smallp.tile([B, 512], FP32, tag="xa", name=f"xa_{j}")
            nc.scalar.activation(xa[:], xp[:], AF.Abs)
            nc.scalar.activation(xa[:], xa[:], AF.Exp, scale=-1.0)
            nc.scalar.activation(xa[:], xa[:], AF.Ln, bias=1.0)
            xr = smallp.tile([B, 512], FP32, tag="xr", name=f"xr_{j}")
            nc.scalar.activation(xr[:], xp[:], AF.Relu)
            nc.vector.tensor_add(x1_sb[:, sl], xr[:], xa[:])
        nc.sync.dma_start(x1_o[:], x1_sb[:])

    nc.compile()
    return nc


def make_in_maps(inputs, N=N_FULL, B=B_FULL, H=H_FULL, R=R_FULL,
                 ncores=NCORES):
    """Host-side prep: fold scalars/identity into the streamed matrices,
    cast to bf16, reshape p-major, and shard across cores."""
    LOC = N // ncores
    KT = N // 128
    f32 = np.float32

    def softplus(x):
        return np.log1p(np.exp(np.float64(x)))

    def pmajor(a):
        # [N, C] -> [128, KT, C] with element (p, k, c) = a[k*128+p, c]
        return np.ascontiguousarray(
            a.reshape(KT, 128, -1).transpose(1, 0, 2))

    def b16(a):
        return np.ascontiguousarray(np.asarray(a).astype(NP_BF16))

    a_s = f32(DT * (softplus(inputs["raw_cs"]) + 1e-4))
    a_f = f32(DT * (softplus(inputs["raw_cf"]) + 1e-4))
    b_s = f32(DT * (softplus(inputs["raw_lambda_s"]) + 1e-4))
    b_f = f32(DT * (softplus(inputs["raw_lambda_f"]) + 1e-4))

    Ls = np.asarray(inputs["Ls"], f32)
    Lf = np.asarray(inputs["Lf"], f32)
    x0 = np.asarray(inputs["x0"], f32)

    com = {
        "x0T": b16(x0.T),
        "MsB": b16(inputs["Ms_B"]),
        "MfB": b16(inputs["Mf_B"]),
        "W1r": pmajor(np.asarray(inputs["W1"], f32)).astype(NP_BF16),
        "D1r": pmajor(np.asarray(inputs["D1"], f32)).astype(NP_BF16),
        "b1r": b16(np.asarray(inputs["b1"], f32).reshape(1, H)),
        "db1r": b16(np.asarray(inputs["db1"], f32).reshape(1, H)),
        "ones8": np.ones((1, B), NP_BF16),
        "eye8": np.eye(B, dtype=NP_BF16),
    }
    W2 = np.asarray(inputs["W2"], f32)
    D2 = np.asarray(inputs["D2"], f32)
    b2 = np.asarray(inputs["b2"], f32)
    db2 = np.asarray(inputs["db2"], f32)
    MsA = np.asarray(inputs["Ms_A"], f32)
    MfA = np.asarray(inputs["Mf_A"], f32)

    in_maps = []
    diag = np.arange(LOC)
    for c in range(ncores):
        r0, r1 = c * LOC, (c + 1) * LOC
        As_c = (-a_s) * Ls[r0:r1, :].T          # [N, LOC]
        As_c[r0 + diag, diag] += f32(1.0)
        Af_c = (-a_f) * Lf[r0:r1, :].T
        Af_c[r0 + diag, diag] += f32(1.0)
        m = dict(com)
        m.update({
            "As": pmajor(As_c).astype(NP_BF16),
            "Af": pmajor(Af_c).astype(NP_BF16),
            "x0lb": np.ascontiguousarray(x0[:, r0:r1]),
            "MsAT": b16((b_s * MsA[r0:r1]).T),
            "MfAT": b16((b_f * MfA[r0:r1]).T),
            "W2": b16(W2[:, r0:r1]),
            "b2r": b16(b2[r0:r1].reshape(1, LOC)),
            "D2": b16(D2[:, r0:r1]),
            "db2r": b16(db2[r0:r1].reshape(1, LOC)),
        })
        in_maps.append(m)
    return in_maps, (a_s, a_f, b_s, b_f)


def assemble_outputs(results, ncores=NCORES):
    f32 = np.float32
    x1 = np.concatenate([np.asarray(results[c]["x1_o"], f32)
                         for c in range(ncores)], axis=1)
    us = np.concatenate([np.asarray(results[c]["us_o"])
                         for c in range(ncores)], axis=0).T.astype(f32)
    uf = np.concatenate([np.asarray(results[c]["uf_o"])
                         for c in range(ncores)], axis=0).T.astype(f32)
    return (np.ascontiguousarray(x1), np.ascontiguousarray(us),
            np.ascontiguousarray(uf))


_PROGRAM_CACHE = {}


def kernel(**inputs):
    """Full-input / full-output entry point for the harness."""
    in_maps, _scal = make_in_maps(inputs)
    key = "full"
    if key not in _PROGRAM_CACHE:
        _PROGRAM_CACHE[key] = build_program()
    nc = _PROGRAM_CACHE[key]

    res = run_bass_kernel_spmd(nc, in_maps, core_ids=list(range(NCORES)))
    return assemble_outputs(res.results)


# revision 6
# speedup vs baseline: 3.8617x; 1.1150x over previous
"""Trainium2 Bass kernel for the CoupledTauModel (gnn_message_passing).

Strategy (v2 — flipped matmul orientation, bf16)
------------------------------------------------
The Euler recurrence  us' = relu(us@(I-a_s*Ls).T + b_s*(uf@Ms_B)@Ms_A.T)
is computed with the *state* as the 128x8 stationary operand and the folded
matrix A = I - a*L streamed from HBM as the bf16 *moving* operand in 512-col
chunks, accumulating PSUM tiles of shape [B=8, 512].  This replaces the v1
orientation (A stationary, 8-wide moving state) which paid a 128x128
LDWEIGHTS per 8 moving columns and ran fp32 (2 HW passes per matmul).

Sharding: core c owns output columns [c*LOC, (c+1)*LOC) of A (column/output
-node sharding).  After each phase the new local state slice [8, LOC] is
transposed on the PE (8x [8,128] -> [128,8]) back into node-major layout and
AllGathered per state, so each collective hides under the other phase's
compute.

Everything on the matmul path is bf16 (verified 8.3e-3 worst rel err vs the
fp32 reference, under the 2e-2 gate); PSUM accumulation stays fp32.
"""

import math
from contextlib import ExitStack

import numpy as np
import ml_dtypes

import concourse.bass as bass
import concourse.mybir as mybir
import concourse.tile as tile
from concourse import bacc
from concourse.bass_utils import run_bass_kernel_spmd

AF = mybir.ActivationFunctionType
FP32 = mybir.dt.float32
BF16 = mybir.dt.bfloat16
NP_BF16 = ml_dtypes.bfloat16

# Full-size problem config (hardcoded; the harness always uses this shape).
N_FULL = 8192
B_FULL = 8
H_FULL = 128
R_FULL = 5
NCORES = 8
NSTEPS_FULL = 10
DT = 0.1


def build_program(N=N_FULL, B=B_FULL, H=H_FULL, R=R_FULL, ncores=NCORES,
                  nsteps=NSTEPS_FULL, kchunk=8, slab_bufs=3, pin_chunks=3,
                  do_ag=True):
    """Build the SPMD Bass program (identical on every core; per-core data
    comes from each core's input map)."""
    LOC = N // ncores          # output columns owned per core
    KT = N // 128              # contraction k-tiles
    MT = LOC // 128            # 128-row tiles per local slice
    NJ = LOC // 512            # psum output chunks per phase
    NCH = KT // kchunk         # streamed slab chunks per phase
    assert N % (128 * ncores) == 0 and H == 128 and LOC % 512 == 0
    assert KT % kchunk == 0 and pin_chunks <= NCH

    nc = bacc.Bacc("TRN2", target_bir_lowering=False, debug=False,
                   num_devices=ncores)

    # ---- DRAM inputs (bf16 unless noted) ----
    As_d = nc.dram_tensor("As", [128, KT, LOC], BF16, kind="ExternalInput")
    Af_d = nc.dram_tensor("Af", [128, KT, LOC], BF16, kind="ExternalInput")
    x0T_d = nc.dram_tensor("x0T", [N, B], BF16, kind="ExternalInput")
    x0lb_d = nc.dram_tensor("x0lb", [B, LOC], FP32, kind="ExternalInput")
    MsB_d = nc.dram_tensor("MsB", [N, R], BF16, kind="ExternalInput")
    MfB_d = nc.dram_tensor("MfB", [N, R], BF16, kind="ExternalInput")
    MsAT_d = nc.dram_tensor("MsAT", [R, LOC], BF16, kind="ExternalInput")
    MfAT_d = nc.dram_tensor("MfAT", [R, LOC], BF16, kind="ExternalInput")
    W1_d = nc.dram_tensor("W1r", [128, KT, H], BF16, kind="ExternalInput")
    W2_d = nc.dram_tensor("W2", [H, LOC], BF16, kind="ExternalInput")
    D1_d = nc.dram_tensor("D1r", [128, KT, H], BF16, kind="ExternalInput")
    D2_d = nc.dram_tensor("D2", [H, LOC], BF16, kind="ExternalInput")
    b1r_d = nc.dram_tensor("b1r", [1, H], BF16, kind="ExternalInput")
    b2r_d = nc.dram_tensor("b2r", [1, LOC], BF16, kind="ExternalInput")
    db1r_d = nc.dram_tensor("db1r", [1, H], BF16, kind="ExternalInput")
    db2r_d = nc.dram_tensor("db2r", [1, LOC], BF16, kind="ExternalInput")
    ones_d = nc.dram_tensor("ones8", [1, B], BF16, kind="ExternalInput")
    eye_d = nc.dram_tensor("eye8", [B, B], BF16, kind="ExternalInput")

    x1_o = nc.dram_tensor("x1_o", [B, LOC], FP32, kind="ExternalOutput")
    us_o = nc.dram_tensor("us_o", [LOC, B], BF16, kind="ExternalOutput")
    uf_o = nc.dram_tensor("uf_o", [LOC, B], BF16, kind="ExternalOutput")

    RG = [list(range(ncores))]

    with ExitStack() as ctx:
        tc = ctx.enter_context(tile.TileContext(nc))
        const = ctx.enter_context(tc.tile_pool(name="const", bufs=1))
        statep = ctx.enter_context(tc.tile_pool(name="state", bufs=3))
        slabp = ctx.enter_context(tc.tile_pool(name="slab", bufs=slab_bufs))
        smallp = ctx.enter_context(tc.tile_pool(name="small", bufs=2))
        newp = ctx.enter_context(tc.tile_pool(name="new", bufs=2))
        psum = ctx.enter_context(tc.tile_pool(name="psum", bufs=1, space="PSUM"))
        dram = ctx.enter_context(tc.tile_pool(name="dram", bufs=2, space="DRAM"))

        # ---- resident constants ----
        x0T_sb = const.tile([128, KT, B], BF16, tag="x0T")
        nc.gpsimd.dma_start(x0T_sb[:], x0T_d[:].rearrange("(k p) b -> p k b", p=128))
        x0lb_sb = const.tile([B, LOC], FP32, tag="x0lb")
        nc.gpsimd.dma_start(x0lb_sb[:], x0lb_d[:])
        MsB_sb = const.tile([128, KT, R], BF16, tag="MsB")
        nc.gpsimd.dma_start(MsB_sb[:], MsB_d[:].rearrange("(k p) r -> p k r", p=128))
        MfB_sb = const.tile([128, KT, R], BF16, tag="MfB")
        nc.gpsimd.dma_start(MfB_sb[:], MfB_d[:].rearrange("(k p) r -> p k r", p=128))
        MsAT_sb = const.tile([R, LOC], BF16, tag="MsAT")
        nc.gpsimd.dma_start(MsAT_sb[:], MsAT_d[:])
        MfAT_sb = const.tile([R, LOC], BF16, tag="MfAT")
        nc.gpsimd.dma_start(MfAT_sb[:], MfAT_d[:])
        W2_sb = const.tile([H, LOC], BF16, tag="W2")
        nc.gpsimd.dma_start(W2_sb[:], W2_d[:])
        D2_sb = const.tile([H, LOC], BF16, tag="D2")
        nc.gpsimd.dma_start(D2_sb[:], D2_d[:])
        b1r_sb = const.tile([1, H], BF16, tag="b1r")
        nc.gpsimd.dma_start(b1r_sb[:], b1r_d[:])
        b2r_sb = const.tile([1, LOC], BF16, tag="b2r")
        nc.gpsimd.dma_start(b2r_sb[:], b2r_d[:])
        db1r_sb = const.tile([1, H], BF16, tag="db1r")
        nc.gpsimd.dma_start(db1r_sb[:], db1r_d[:])
        db2r_sb = const.tile([1, LOC], BF16, tag="db2r")
        nc.gpsimd.dma_start(db2r_sb[:], db2r_d[:])
        ones_sb = const.tile([1, B], BF16, tag="ones")
        nc.gpsimd.dma_start(ones_sb[:], ones_d[:])
        eye_sb = const.tile([B, B], BF16, tag="eye")
        nc.gpsimd.dma_start(eye_sb[:], eye_d[:])

        # ---- pinned A chunks: loaded once, reused every step ----
        pinned = [[], []]
        if pin_chunks:
            pinp = ctx.enter_context(tc.tile_pool(name="pin", bufs=1))
            for ph, A_d in enumerate([As_d, Af_d]):
                for c in range(pin_chunks):
                    pt_ = pinp.tile([128, kchunk, LOC], BF16,
                                    tag=f"pin{ph}_{c}", name=f"pin{ph}_{c}")
                    nc.sync.dma_start(
                        pt_[:], A_d[:, c * kchunk:(c + 1) * kchunk, :])
                    pinned[ph].append(pt_)

        def bT_to_nT(src_bT, nm):
            """[B, LOC] bf16 SBUF -> node-major [128, MT, B] bf16 tile via PE
            transposes."""
            news = newp.tile([128, MT, B], BF16, tag="news", name=f"news_{nm}")
            for m in range(MT):
                pt = psum.tile([128, B], BF16, tag="pt", bufs=2,
                               name=f"pt_{nm}_{m}")
                nc.tensor.transpose(pt[:], src_bT[:, m * 128:(m + 1) * 128],
                                    eye_sb[:])
                nc.vector.tensor_copy(news[:, m, :], pt[:])
            return news

        def exchange(news, nm):
            """AllGather one state's local [LOC, B] update -> full [128,KT,B]
            stationary tile."""
            agin = dram.tile([LOC, B], BF16, tag=f"agin_{nm[-1]}",
                             name=f"agin_{nm}")
            nc.scalar.dma_start(
                agin[:].rearrange("(m p) b -> p m b", p=128), news[:])
            agout = dram.tile([N, B], BF16, tag=f"agout_{nm[-1]}",
                              name=f"agout_{nm}")
            if do_ag:
                nc.gpsimd.collective_compute(
                    "AllGather", mybir.AluOpType.bypass, replica_groups=RG,
                    ins=[agin[:]], outs=[agout[:]])
            else:
                nc.gpsimd.dma_start(agout[0:LOC, :], agin[:])
            st = statep.tile([128, KT, B], BF16, tag=f"st_{nm[-1]}",
                             name=f"st_{nm}")
            nc.gpsimd.dma_start(
                st[:], agout[:].rearrange("(k p) b -> p k b", p=128))
            return agin, st

        # ---- gate MLP:  gate = sigmoid(relu(x0@W1+b1)@W2+b2) ----
        W1_sb = slabp.tile([128, KT, H], BF16, tag="slab", name="W1_sb")
        nc.scalar.dma_start(W1_sb[:], W1_d[:])
        hp = psum.tile([B, H], FP32, tag="q")
        for k in range(KT):
            nc.tensor.matmul(hp[:], x0T_sb[:, k, :], W1_sb[:, k, :],
                             start=(k == 0), stop=False)
        nc.tensor.matmul(hp[:], ones_sb[:], b1r_sb[:], start=False, stop=True)
        h_sb = smallp.tile([B, H], BF16, tag="h")
        nc.scalar.activation(h_sb[:], hp[:], AF.Relu)
        htp = psum.tile([128, B], BF16, tag="pt", bufs=2)
        nc.tensor.transpose(htp[:], h_sb[:], eye_sb[:])
        hT_sb = smallp.tile([H, B], BF16, tag="hT")
        nc.vector.tensor_copy(hT_sb[:], htp[:])

        gate_sb = smallp.tile([B, LOC], FP32, tag="gate", bufs=1)
        for j in range(NJ):
            gp = psum.tile([B, 512], FP32, tag=f"out{j}", bufs=2,
                           name=f"gp_{j}")
            nc.tensor.matmul(gp[:], hT_sb[:], W2_sb[:, j * 512:(j + 1) * 512],
                             start=True, stop=False)
            nc.tensor.matmul(gp[:], ones_sb[:],
                             b2r_sb[:, j * 512:(j + 1) * 512],
                             start=False, stop=True)
            nc.scalar.activation(gate_sb[:, j * 512:(j + 1) * 512], gp[:],
                                 AF.Sigmoid)
        us0_b = smallp.tile([B, LOC], FP32, tag="us0b", bufs=1)
        nc.vector.tensor_mul(us0_b[:], gate_sb[:], x0lb_sb[:])
        uf0_b = smallp.tile([B, LOC], FP32, tag="uf0b", bufs=1)
        nc.vector.tensor_sub(uf0_b[:], x0lb_sb[:], us0_b[:])
        us0_bT = smallp.tile([B, LOC], BF16, tag="new_bT", bufs=2)
        nc.vector.tensor_copy(us0_bT[:], us0_b[:])
        uf0_bT = smallp.tile([B, LOC], BF16, tag="new_bT", bufs=2)
        nc.vector.tensor_copy(uf0_bT[:], uf0_b[:])

        news0 = bT_to_nT(us0_bT, "s0")
        newf0 = bT_to_nT(uf0_bT, "f0")
        _, us_cur = exchange(news0, "us0")
        _, uf_cur = exchange(newf0, "uf0")

        # ---- Euler steps ----
        last_agin = [None, None]
        for t in range(nsteps):
            new_st = [None, None]
            for ph in range(2):
                A_d = As_d if ph == 0 else Af_d
                stA = us_cur if ph == 0 else uf_cur      # stationary state
                stQ = uf_cur if ph == 0 else us_cur      # rank-term state
                qB = MsB_sb if ph == 0 else MfB_sb
                MAT = MsAT_sb if ph == 0 else MfAT_sb
                nm = f"{t}_{'s' if ph == 0 else 'f'}"

                outs = [psum.tile([B, 512], FP32, tag=f"out{j}", bufs=2,
                                  name=f"out{nm}_{j}") for j in range(NJ)]
                # pinned chunks first: PE has immediate work while the
                # streamed slabs arrive from HBM
                npin = len(pinned[ph])
                chunk_order = list(range(npin)) + list(range(npin, NCH))
                for ci, c in enumerate(chunk_order):
                    if c < npin:
                        slab = pinned[ph][c]
                    else:
                        slab = slabp.tile([128, kchunk, LOC], BF16, tag="slab",
                                          name=f"slab_{nm}_{c}")
                        nc.sync.dma_start(
                            slab[:], A_d[:, c * kchunk:(c + 1) * kchunk, :])
                    for kk in range(kchunk):
                        k = c * kchunk + kk
                        for j in range(NJ):
                            nc.tensor.matmul(
                                outs[j][:], stA[:, k, :],
                                slab[:, kk, j * 512:(j + 1) * 512],
                                start=(ci == 0 and kk == 0), stop=False)
                # rank-R term: q = state @ M_B  (accumulated), then transpose
                qp = psum.tile([B, R], FP32, tag="q", name=f"qp_{nm}")
                for k in range(KT):
                    nc.tensor.matmul(qp[:], stQ[:, k, :], qB[:, k, :],
                                     start=(k == 0), stop=(k == KT - 1))
                q_sb = smallp.tile([B, R], BF16, tag="q", name=f"q_{nm}")
                nc.vector.tensor_copy(q_sb[:], qp[:])
                qtp = psum.tile([R, B], BF16, tag="qt", name=f"qtp_{nm}")
                nc.tensor.transpose(qtp[:], q_sb[:], eye_sb[:])
                qT_sb = smallp.tile([R, B], BF16, tag="qT", name=f"qT_{nm}")
                nc.vector.tensor_copy(qT_sb[:], qtp[:])
                for j in range(NJ):
                    nc.tensor.matmul(outs[j][:], qT_sb[:],
                                     MAT[:, j * 512:(j + 1) * 512],
                                     start=False, stop=True)
                new_bT = smallp.tile([B, LOC], BF16, tag="new_bT", bufs=2,
                                     name=f"newbT_{nm}")
                for j in range(NJ):
                    nc.scalar.activation(new_bT[:, j * 512:(j + 1) * 512],
                                         outs[j][:], AF.Relu)
                news = bT_to_nT(new_bT, nm)
                last_agin[ph], new_st[ph] = exchange(news, nm)
            us_cur, uf_cur = new_st

        # final local state -> outputs (DRAM->DRAM copy out of the AG input)
        nc.gpsimd.dma_start(us_o[:], last_agin[0][:])
        nc.gpsimd.dma_start(uf_o[:], last_agin[1][:])

        # ---- decoder:  x1 = softplus(relu((us+uf)@D1+db1)@D2+db2) ----
        lat = statep.tile([128, KT, B], BF16, tag="lat", bufs=1)
        nc.vector.tensor_add(lat[:], us_cur[:], uf_cur[:])
        D1_sb = slabp.tile([128, KT, H], BF16, tag="slab", name="D1_sb")
        nc.scalar.dma_start(D1_sb[:], D1_d[:])
        hdp = psum.tile([B, H], FP32, tag="q")
        for k in range(KT):
            nc.tensor.matmul(hdp[:], lat[:, k, :], D1_sb[:, k, :],
                             start=(k == 0), stop=False)
        nc.tensor.matmul(hdp[:], ones_sb[:], db1r_sb[:], start=False, stop=True)
        hd_sb = smallp.tile([B, H], BF16, tag="h")
        nc.scalar.activation(hd_sb[:], hdp[:], AF.Relu)
        hdtp = psum.tile([128, B], BF16, tag="pt", bufs=2)
        nc.tensor.transpose(hdtp[:], hd_sb[:], eye_sb[:])
        hdT_sb = smallp.tile([H, B], BF16, tag="hT")
        nc.vector.tensor_copy(hdT_sb[:], hdtp[:])

        x1_sb = smallp.tile([B, LOC], FP32, tag="x1", bufs=1)
        for j in range(NJ):
            xp = psum.tile([B, 512], FP32, tag=f"out{j}", bufs=2,
                           name=f"xp_{j}")
            nc.tensor.matmul(xp[:], hdT_sb[:], D2_sb[:, j * 512:(j + 1) * 512],
                             start=True, stop=False)
            nc.tensor.matmul(xp[:], ones_sb[:],
                             db2r_sb[:, j * 512:(j + 1) * 512],
                             start=False, stop=True)
            # stable softplus(y) = relu(y) + ln(1+exp(-|y|))
            sl = slice(j * 512, (j + 1) * 512)
            xa = smallp.tile([B, 512], FP32, tag="xa", name=f"xa_{j}")
            nc.scalar.activation(xa[:], xp[:], AF.Abs)
            nc.scalar.activation(xa[:], xa[:], AF.Exp, scale=-1.0)
            nc.scalar.activation(xa[:], xa[:], AF.Ln, bias=1.0)
            xr = smallp.tile([B, 512], FP32, tag="xr", name=f"xr_{j}")
            nc.scalar.activation(xr[:], xp[:], AF.Relu)
            nc.vector.tensor_add(x1_sb[:, sl], xr[:], xa[:])
        nc.scalar.dma_start(x1_o[:], x1_sb[:])

    nc.compile()
    return nc


def make_in_maps(inputs, N=N_FULL, B=B_FULL, H=H_FULL, R=R_FULL,
                 ncores=NCORES):
    """Host-side prep: fold scalars/identity into the streamed matrices,
    cast to bf16, reshape p-major, and shard across cores."""
    LOC = N // ncores
    KT = N // 128
    f32 = np.float32

    def softplus(x):
        return np.log1p(np.exp(np.float64(x)))

    def pmajor(a):
        # [N, C] -> [128, KT, C] with element (p, k, c) = a[k*128+p, c]
        return np.ascontiguousarray(
            a.reshape(KT, 128, -1).transpose(1, 0, 2))

    def b16(a):
        return np.ascontiguousarray(np.asarray(a).astype(NP_BF16))

    a_s = f32(DT * (softplus(inputs["raw_cs"]) + 1e-4))
    a_f = f32(DT * (softplus(inputs["raw_cf"]) + 1e-4))
    b_s = f32(DT * (softplus(inputs["raw_lambda_s"]) + 1e-4))
    b_f = f32(DT * (softplus(inputs["raw_lambda_f"]) + 1e-4))

    Ls = np.asarray(inputs["Ls"], f32)
    Lf = np.asarray(inputs["Lf"], f32)
    x0 = np.asarray(inputs["x0"], f32)

    com = {
        "x0T": b16(x0.T),
        "MsB": b16(inputs["Ms_B"]),
        "MfB": b16(inputs["Mf_B"]),
        "W1r": pmajor(np.asarray(inputs["W1"], f32)).astype(NP_BF16),
        "D1r": pmajor(np.asarray(inputs["D1"], f32)).astype(NP_BF16),
        "b1r": b16(np.asarray(inputs["b1"], f32).reshape(1, H)),
        "db1r": b16(np.asarray(inputs["db1"], f32).reshape(1, H)),
        "ones8": np.ones((1, B), NP_BF16),
        "eye8": np.eye(B, dtype=NP_BF16),
    }
    W2 = np.asarray(inputs["W2"], f32)
    D2 = np.asarray(inputs["D2"], f32)
    b2 = np.asarray(inputs["b2"], f32)
    db2 = np.asarray(inputs["db2"], f32)
    MsA = np.asarray(inputs["Ms_A"], f32)
    MfA = np.asarray(inputs["Mf_A"], f32)

    in_maps = []
    diag = np.arange(LOC)
    for c in range(ncores):
        r0, r1 = c * LOC, (c + 1) * LOC
        As_c = (-a_s) * Ls[r0:r1, :].T          # [N, LOC]
        As_c[r0 + diag, diag] += f32(1.0)
        Af_c = (-a_f) * Lf[r0:r1, :].T
        Af_c[r0 + diag, diag] += f32(1.0)
        m = dict(com)
        m.update({
            "As": pmajor(As_c).astype(NP_BF16),
            "Af": pmajor(Af_c).astype(NP_BF16),
            "x0lb": np.ascontiguousarray(x0[:, r0:r1]),
            "MsAT": b16((b_s * MsA[r0:r1]).T),
            "MfAT": b16((b_f * MfA[r0:r1]).T),
            "W2": b16(W2[:, r0:r1]),
            "b2r": b16(b2[r0:r1].reshape(1, LOC)),
            "D2": b16(D2[:, r0:r1]),
            "db2r": b16(db2[r0:r1].reshape(1, LOC)),
        })
        in_maps.append(m)
    return in_maps, (a_s, a_f, b_s, b_f)


def assemble_outputs(results, ncores=NCORES):
    f32 = np.float32
    x1 = np.concatenate([np.asarray(results[c]["x1_o"], f32)
                         for c in range(ncores)], axis=1)
    us = np.concatenate([np.asarray(results[c]["us_o"])
                         for c in range(ncores)], axis=0).T.astype(f32)
    uf = np.concatenate([np.asarray(results[c]["uf_o"])
                         for c in range(ncores)], axis=0).T.astype(f32)
    return (np.ascontiguousarray(x1), np.ascontiguousarray(us),
            np.ascontiguousarray(uf))


_PROGRAM_CACHE = {}


def kernel(**inputs):
    """Full-input / full-output entry point for the harness."""
    in_maps, _scal = make_in_maps(inputs)
    key = "full"
    if key not in _PROGRAM_CACHE:
        _PROGRAM_CACHE[key] = build_program()
    nc = _PROGRAM_CACHE[key]

    res = run_bass_kernel_spmd(nc, in_maps, core_ids=list(range(NCORES)))
    return assemble_outputs(res.results)


# revision 9
# speedup vs baseline: 4.3628x; 1.1298x over previous
"""Trainium2 Bass kernel for the CoupledTauModel (gnn_message_passing).

Strategy (v4 — flipped matmul orientation, bf16, SBUF pinning)
--------------------------------------------------------------
The Euler recurrence  us' = relu(us@(I-a_s*Ls).T + b_s*(uf@Ms_B)@Ms_A.T)
is computed with the *state* as the 128x8 stationary operand and the folded
matrix A = I - a*L streamed from HBM as the bf16 *moving* operand in 512-col
chunks, accumulating PSUM tiles of shape [B=8, 512].

Sharding: core c owns output columns [c*LOC, (c+1)*LOC) of A (column/output
-node sharding).  After each phase the new local state slice [8, LOC] is
relu'd on the DVE, transposed on the PE (8x [8,128] -> [128,8]) back into
node-major layout and AllGathered per state, so each collective hides under
the other phase's compute.

Queue discipline: A-slab streams on the Sync HWDGE queue; exchange DMAs
(AG staging + state reload) live alone on the Scalar HWDGE queue so their
semaphore waits block nothing; GpSimd triggers only collectives (SWDGE
descriptor generation is ~30us for the strided reload, so it must stay on
HWDGE).  Half the A chunks are pinned in SBUF and interleaved with streamed
chunks so the PE (which consumes A at ~614 GB/s) never starves on HBM
(~358 GB/s) and HAM stays un-throttled.

Both rank-R projections are computed in one PSUM group per step:
stationary [us_k | uf_k] (16 cols) x moving [MsB_k | MfB_k] (10 cols) ->
[16, 10], whose off-diagonal blocks are qf and qs.

Everything on the matmul path is bf16 (verified 8.3e-3 worst rel err vs the
fp32 reference, under the 2e-2 gate); PSUM accumulation stays fp32.
"""

import math
from contextlib import ExitStack

import numpy as np
import ml_dtypes

import concourse.bass as bass
import concourse.mybir as mybir
import concourse.tile as tile
from concourse import bacc
from concourse.bass_utils import run_bass_kernel_spmd

AF = mybir.ActivationFunctionType
FP32 = mybir.dt.float32
BF16 = mybir.dt.bfloat16
NP_BF16 = ml_dtypes.bfloat16

# Full-size problem config (hardcoded; the harness always uses this shape).
N_FULL = 8192
B_FULL = 8
H_FULL = 128
R_FULL = 5
NCORES = 8
NSTEPS_FULL = 10
DT = 0.1


def build_program(N=N_FULL, B=B_FULL, H=H_FULL, R=R_FULL, ncores=NCORES,
                  nsteps=NSTEPS_FULL, kchunk=4, slab_bufs=4, pin_chunks=8,
                  do_ag=True):
    """Build the SPMD Bass program (identical on every core; per-core data
    comes from each core's input map)."""
    LOC = N // ncores          # output columns owned per core
    KT = N // 128              # contraction k-tiles
    MT = LOC // 128            # 128-row tiles per local slice
    NJ = LOC // 512            # psum output chunks per phase
    NCH = KT // kchunk         # A chunks per phase
    KH = KT // 2               # half of the k-tiles (W1/D1 staging)
    assert N % (128 * ncores) == 0 and H == 128 and LOC % 512 == 0
    assert KT % kchunk == 0 and pin_chunks <= NCH and KT % 2 == 0

    nc = bacc.Bacc("TRN2", target_bir_lowering=False, debug=False,
                   num_devices=ncores)

    # ---- DRAM inputs (bf16 unless noted) ----
    As_d = nc.dram_tensor("As", [128, KT, LOC], BF16, kind="ExternalInput")
    Af_d = nc.dram_tensor("Af", [128, KT, LOC], BF16, kind="ExternalInput")
    x0T_d = nc.dram_tensor("x0T", [N, B], BF16, kind="ExternalInput")
    x0lb_d = nc.dram_tensor("x0lb", [B, LOC], FP32, kind="ExternalInput")
    QW = 32 + R               # [MsB | zeros | MfB] padded stationary width
    MB_d = nc.dram_tensor("MBc", [N, QW], BF16, kind="ExternalInput")
    MsAT_d = nc.dram_tensor("MsAT", [R, LOC], BF16, kind="ExternalInput")
    MfAT_d = nc.dram_tensor("MfAT", [R, LOC], BF16, kind="ExternalInput")
    W1_d = nc.dram_tensor("W1r", [128, KT, H], BF16, kind="ExternalInput")
    W2_d = nc.dram_tensor("W2", [H, LOC], BF16, kind="ExternalInput")
    D1_d = nc.dram_tensor("D1r", [128, KT, H], BF16, kind="ExternalInput")
    D2_d = nc.dram_tensor("D2", [H, LOC], BF16, kind="ExternalInput")
    b1r_d = nc.dram_tensor("b1r", [1, H], BF16, kind="ExternalInput")
    b2r_d = nc.dram_tensor("b2r", [1, LOC], BF16, kind="ExternalInput")
    db1r_d = nc.dram_tensor("db1r", [1, H], BF16, kind="ExternalInput")
    db2r_d = nc.dram_tensor("db2r", [1, LOC], BF16, kind="ExternalInput")
    ones_d = nc.dram_tensor("ones8", [1, B], BF16, kind="ExternalInput")
    eye_d = nc.dram_tensor("eye8", [B, B], BF16, kind="ExternalInput")

    x1_o = nc.dram_tensor("x1_o", [B, LOC], FP32, kind="ExternalOutput")
    us_o = nc.dram_tensor("us_o", [LOC, B], BF16, kind="ExternalOutput")
    uf_o = nc.dram_tensor("uf_o", [LOC, B], BF16, kind="ExternalOutput")

    RG = [list(range(ncores))]

    with ExitStack() as ctx:
        tc = ctx.enter_context(tile.TileContext(nc))
        const = ctx.enter_context(tc.tile_pool(name="const", bufs=1))
        statep = ctx.enter_context(tc.tile_pool(name="state", bufs=3))
        slabp = ctx.enter_context(tc.tile_pool(name="slab", bufs=slab_bufs))
        smallp = ctx.enter_context(tc.tile_pool(name="small", bufs=1))
        newp = ctx.enter_context(tc.tile_pool(name="new", bufs=2))
        psum = ctx.enter_context(tc.tile_pool(name="psum", bufs=1, space="PSUM"))
        dram = ctx.enter_context(tc.tile_pool(name="dram", bufs=2, space="DRAM"))

        # ---- resident constants (scalar HWDGE queue) ----
        x0T_sb = const.tile([128, KT, B], BF16, tag="x0T")
        nc.scalar.dma_start(x0T_sb[:], x0T_d[:].rearrange("(k p) b -> p k b", p=128))
        x0lb_sb = const.tile([B, LOC], FP32, tag="x0lb")
        nc.scalar.dma_start(x0lb_sb[:], x0lb_d[:])
        MB_sb = const.tile([128, KT, QW], BF16, tag="MBc")
        nc.scalar.dma_start(MB_sb[:], MB_d[:].rearrange("(k p) r -> p k r", p=128))
        MsAT_sb = const.tile([R, LOC], BF16, tag="MsAT")
        nc.scalar.dma_start(MsAT_sb[:], MsAT_d[:])
        MfAT_sb = const.tile([R, LOC], BF16, tag="MfAT")
        nc.scalar.dma_start(MfAT_sb[:], MfAT_d[:])
        W2_sb = const.tile([H, LOC], BF16, tag="W2")
        nc.scalar.dma_start(W2_sb[:], W2_d[:])
        D2_sb = const.tile([H, LOC], BF16, tag="D2")
        nc.scalar.dma_start(D2_sb[:], D2_d[:])
        b1r_sb = const.tile([1, H], BF16, tag="b1r")
        nc.scalar.dma_start(b1r_sb[:], b1r_d[:])
        b2r_sb = const.tile([1, LOC], BF16, tag="b2r")
        nc.scalar.dma_start(b2r_sb[:], b2r_d[:])
        db1r_sb = const.tile([1, H], BF16, tag="db1r")
        nc.scalar.dma_start(db1r_sb[:], db1r_d[:])
        db2r_sb = const.tile([1, LOC], BF16, tag="db2r")
        nc.scalar.dma_start(db2r_sb[:], db2r_d[:])
        ones_sb = const.tile([1, B], BF16, tag="ones")
        nc.scalar.dma_start(ones_sb[:], ones_d[:])
        eye_sb = const.tile([B, B], BF16, tag="eye")
        nc.scalar.dma_start(eye_sb[:], eye_d[:])

        # ---- pinned A chunks: loaded once, reused every step ----
        pinned = [{}, {}]
        if pin_chunks:
            pinp = ctx.enter_context(tc.tile_pool(name="pin", bufs=1))
            for ph, A_d in enumerate([As_d, Af_d]):
                for i in range(pin_chunks):
                    # pin every other chunk so pinned/streamed interleave
                    c = min(2 * i, NCH - 1)
                    pt_ = pinp.tile([128, kchunk, LOC], BF16,
                                    tag=f"pin{ph}_{i}", name=f"pin{ph}_{i}")
                    nc.sync.dma_start(
                        pt_[:], A_d[:, c * kchunk:(c + 1) * kchunk, :])
                    pinned[ph][c] = pt_

        def bT_to_nT(src_bT, nm):
            """[B, LOC] bf16 SBUF -> node-major [128, MT, B] bf16 tile via PE
            transposes."""
            news = newp.tile([128, MT, B], BF16, tag="news", name=f"news_{nm}")
            for m in range(MT):
                pt = psum.tile([128, B], BF16, tag="pt", bufs=2,
                               name=f"pt_{nm}_{m}")
                nc.tensor.transpose(pt[:], src_bT[:, m * 128:(m + 1) * 128],
                                    eye_sb[:])
                nc.vector.tensor_copy(news[:, m, :], pt[:])
            return news

        def exchange(news, nm, dst):
            """AllGather one state's local [LOC, B] update into `dst`, a
            [128, KT, B] slice of the combined state tile.  All DMAs on the
            scalar HWDGE queue."""
            agin = dram.tile([LOC, B], BF16, tag=f"agin_{nm[-1]}",
                             name=f"agin_{nm}")
            nc.scalar.dma_start(
                agin[:].rearrange("(m p) b -> p m b", p=128), news[:])
            agout = dram.tile([N, B], BF16, tag=f"agout_{nm[-1]}",
                              name=f"agout_{nm}")
            if do_ag:
                nc.gpsimd.collective_compute(
                    "AllGather", mybir.AluOpType.bypass, replica_groups=RG,
                    ins=[agin[:]], outs=[agout[:]])
            else:
                nc.gpsimd.dma_start(agout[0:LOC, :], agin[:])
            nc.scalar.dma_start(
                dst, agout[:].rearrange("(k p) b -> p k b", p=128))
            return agin

        # ---- gate MLP:  gate = sigmoid(relu(x0@W1+b1)@W2+b2) ----
        W1a = slabp.tile([128, KH, H], BF16, tag="slab", name="W1a")
        nc.scalar.dma_start(W1a[:], W1_d[:, 0:KH, :])
        W1b = slabp.tile([128, KH, H], BF16, tag="slab", name="W1b")
        nc.scalar.dma_start(W1b[:], W1_d[:, KH:KT, :])
        hp = psum.tile([B, H], FP32, tag="q")
        for k in range(KT):
            w = W1a if k < KH else W1b
            nc.tensor.matmul(hp[:], x0T_sb[:, k, :], w[:, k % KH, :],
                             start=(k == 0), stop=False)
        nc.tensor.matmul(hp[:], ones_sb[:], b1r_sb[:], start=False, stop=True)
        h_sb = smallp.tile([B, H], BF16, tag="h")
        nc.scalar.activation(h_sb[:], hp[:], AF.Relu)
        htp = psum.tile([128, B], BF16, tag="pt", bufs=2)
        nc.tensor.transpose(htp[:], h_sb[:], eye_sb[:])
        hT_sb = smallp.tile([H, B], BF16, tag="hT")
        nc.vector.tensor_copy(hT_sb[:], htp[:])

        us0_bT = smallp.tile([B, LOC], BF16, tag="new_bT", bufs=2)
        uf0_bT = smallp.tile([B, LOC], BF16, tag="new_bT", bufs=2)
        for j in range(NJ):
            sl = slice(j * 512, (j + 1) * 512)
            gp = psum.tile([B, 512], FP32, tag=f"out{j}", bufs=2,
                           name=f"gp_{j}")
            nc.tensor.matmul(gp[:], hT_sb[:], W2_sb[:, sl],
                             start=True, stop=False)
            nc.tensor.matmul(gp[:], ones_sb[:], b2r_sb[:, sl],
                             start=False, stop=True)
            g = smallp.tile([B, 512], FP32, tag="tmpg", name=f"g_{j}")
            nc.scalar.activation(g[:], gp[:], AF.Sigmoid)
            u = smallp.tile([B, 512], FP32, tag="tmpu", name=f"u_{j}")
            nc.vector.tensor_mul(u[:], g[:], x0lb_sb[:, sl])
            nc.vector.tensor_copy(us0_bT[:, sl], u[:])
            nc.vector.tensor_sub(uf0_bT[:, sl], x0lb_sb[:, sl], u[:])

        news0 = bT_to_nT(us0_bT, "s0")
        newf0 = bT_to_nT(uf0_bT, "f0")
        stb = statep.tile([128, KT, 2 * B], BF16, tag="stb", name="stb_0")
        exchange(news0, "us0", stb[:, :, 0:B])
        exchange(newf0, "uf0", stb[:, :, B:2 * B])

        # ---- Euler steps ----
        last_agin = [None, None]
        for t in range(nsteps):
            stb_next = statep.tile([128, KT, 2 * B], BF16, tag="stb",
                                   name=f"stb_{t + 1}")
            qT = [None, None]
            for ph in range(2):
                A_d = As_d if ph == 0 else Af_d
                stA = stb[:, :, ph * B:(ph + 1) * B]     # stationary state
                MAT = MsAT_sb if ph == 0 else MfAT_sb
                nm = f"{t}_{'s' if ph == 0 else 'f'}"

                outs = [psum.tile([B, 512], FP32, tag=f"out{j}", bufs=2,
                                  name=f"out{nm}_{j}") for j in range(NJ)]
                # pinned chunks are interleaved with streamed ones so the PE
                # has immediate work while each streamed slab arrives
                first = True
                for c in range(NCH):
                    if c in pinned[ph]:
                        slab = pinned[ph][c]
                    else:
                        slab = slabp.tile([128, kchunk, LOC], BF16, tag="slab",
                                          name=f"slab_{nm}_{c}")
                        nc.sync.dma_start(
                            slab[:], A_d[:, c * kchunk:(c + 1) * kchunk, :])
                    for kk in range(kchunk):
                        k = c * kchunk + kk
                        for j in range(NJ):
                            nc.tensor.matmul(
                                outs[j][:], stA[:, k, :],
                                slab[:, kk, j * 512:(j + 1) * 512],
                                start=first, stop=False)
                        first = False

                if ph == 0:
                    # both rank-R projections, pre-transposed, in one PSUM
                    # group: [MsB_k | 0 | MfB_k].T @ [us_k | uf_k] = [37, 16]
                    # qsT = block [0:R, 8:16]   ((uf@MsB).T)
                    # qfT = block [32:32+R, 0:8]  ((us@MfB).T)
                    # (MfB sits at column 32 so both PSUM reads are
                    # partition-32-aligned.)
                    qp = psum.tile([QW, 2 * B], FP32, tag="q",
                                   name=f"qp_{t}")
                    for k in range(KT):
                        nc.tensor.matmul(qp[:], MB_sb[:, k, :], stb[:, k, :],
                                         start=(k == 0), stop=(k == KT - 1))
                    qTs = smallp.tile([R, B], BF16, tag="qT0", bufs=2,
                                      name=f"qTs_{t}")
                    nc.vector.tensor_copy(qTs[:], qp[0:R, B:2 * B])
                    qTf = smallp.tile([R, B], BF16, tag="qT1", bufs=2,
                                      name=f"qTf_{t}")
                    nc.vector.tensor_copy(qTf[:], qp[32:32 + R, 0:B])
                    qT = [qTs, qTf]

                for j in range(NJ):
                    nc.tensor.matmul(outs[j][:], qT[ph][:],
                                     MAT[:, j * 512:(j + 1) * 512],
                                     start=False, stop=True)
                new_bT = smallp.tile([B, LOC], BF16, tag="new_bT", bufs=2,
                                     name=f"newbT_{nm}")
                for j in range(NJ):
                    nc.vector.tensor_scalar_max(
                        new_bT[:, j * 512:(j + 1) * 512], outs[j][:], 0.0)
                news = bT_to_nT(new_bT, nm)
                last_agin[ph] = exchange(
                    news, nm, stb_next[:, :, ph * B:(ph + 1) * B])
            stb = stb_next

        # final local state -> outputs (DRAM->DRAM copy out of the AG input)
        nc.scalar.dma_start(us_o[:], last_agin[0][:])
        nc.scalar.dma_start(uf_o[:], last_agin[1][:])

        # ---- decoder:  x1 = softplus(relu((us+uf)@D1+db1)@D2+db2) ----
        lat = statep.tile([128, KT, B], BF16, tag="lat", bufs=1)
        nc.vector.tensor_add(lat[:], stb[:, :, 0:B], stb[:, :, B:2 * B])
        D1a = slabp.tile([128, KH, H], BF16, tag="slab", name="D1a")
        nc.scalar.dma_start(D1a[:], D1_d[:, 0:KH, :])
        D1b = slabp.tile([128, KH, H], BF16, tag="slab", name="D1b")
        nc.scalar.dma_start(D1b[:], D1_d[:, KH:KT, :])
        hdp = psum.tile([B, H], FP32, tag="q")
        for k in range(KT):
            w = D1a if k < KH else D1b
            nc.tensor.matmul(hdp[:], lat[:, k, :], w[:, k % KH, :],
                             start=(k == 0), stop=False)
        nc.tensor.matmul(hdp[:], ones_sb[:], db1r_sb[:], start=False, stop=True)
        hd_sb = smallp.tile([B, H], BF16, tag="h")
        nc.scalar.activation(hd_sb[:], hdp[:], AF.Relu)
        hdtp = psum.tile([128, B], BF16, tag="pt", bufs=2)
        nc.tensor.transpose(hdtp[:], hd_sb[:], eye_sb[:])
        hdT_sb = smallp.tile([H, B], BF16, tag="hT")
        nc.vector.tensor_copy(hdT_sb[:], hdtp[:])

        for j in range(NJ):
            sl = slice(j * 512, (j + 1) * 512)
            xp = psum.tile([B, 512], FP32, tag=f"out{j}", bufs=2,
                           name=f"xp_{j}")
            nc.tensor.matmul(xp[:], hdT_sb[:], D2_sb[:, sl],
                             start=True, stop=False)
            nc.tensor.matmul(xp[:], ones_sb[:], db2r_sb[:, sl],
                             start=False, stop=True)
            # stable softplus(y) = relu(y) + ln(1+exp(-|y|))
            xa = smallp.tile([B, 512], FP32, tag="tmpg", name=f"xa_{j}")
            nc.scalar.activation(xa[:], xp[:], AF.Abs)
            nc.scalar.activation(xa[:], xa[:], AF.Exp, scale=-1.0)
            nc.scalar.activation(xa[:], xa[:], AF.Ln, bias=1.0)
            xr = smallp.tile([B, 512], FP32, tag="tmpu", name=f"xr_{j}")
            nc.scalar.activation(xr[:], xp[:], AF.Relu)
            x1c = smallp.tile([B, 512], FP32, tag="x1c", bufs=2,
                              name=f"x1c_{j}")
            nc.vector.tensor_add(x1c[:], xr[:], xa[:])
            nc.scalar.dma_start(x1_o[:, sl], x1c[:])

    nc.compile()
    return nc


def make_in_maps(inputs, N=N_FULL, B=B_FULL, H=H_FULL, R=R_FULL,
                 ncores=NCORES):
    """Host-side prep: fold scalars/identity into the streamed matrices,
    cast to bf16, reshape p-major, and shard across cores."""
    LOC = N // ncores
    KT = N // 128
    f32 = np.float32

    def softplus(x):
        return np.log1p(np.exp(np.float64(x)))

    def pmajor(a):
        # [N, C] -> [128, KT, C] with element (p, k, c) = a[k*128+p, c]
        return np.ascontiguousarray(
            a.reshape(KT, 128, -1).transpose(1, 0, 2))

    def b16(a):
        return np.ascontiguousarray(np.asarray(a).astype(NP_BF16))

    a_s = f32(DT * (softplus(inputs["raw_cs"]) + 1e-4))
    a_f = f32(DT * (softplus(inputs["raw_cf"]) + 1e-4))
    b_s = f32(DT * (softplus(inputs["raw_lambda_s"]) + 1e-4))
    b_f = f32(DT * (softplus(inputs["raw_lambda_f"]) + 1e-4))

    Ls = np.asarray(inputs["Ls"], f32)
    Lf = np.asarray(inputs["Lf"], f32)
    x0 = np.asarray(inputs["x0"], f32)

    MBc = np.zeros((N, 32 + R), f32)
    MBc[:, 0:R] = np.asarray(inputs["Ms_B"], f32)
    MBc[:, 32:32 + R] = np.asarray(inputs["Mf_B"], f32)

    com = {
        "x0T": b16(x0.T),
        "MBc": b16(MBc),
        "W1r": pmajor(np.asarray(inputs["W1"], f32)).astype(NP_BF16),
        "D1r": pmajor(np.asarray(inputs["D1"], f32)).astype(NP_BF16),
        "b1r": b16(np.asarray(inputs["b1"], f32).reshape(1, H)),
        "db1r": b16(np.asarray(inputs["db1"], f32).reshape(1, H)),
        "ones8": np.ones((1, B), NP_BF16),
        "eye8": np.eye(B, dtype=NP_BF16),
    }
    W2 = np.asarray(inputs["W2"], f32)
    D2 = np.asarray(inputs["D2"], f32)
    b2 = np.asarray(inputs["b2"], f32)
    db2 = np.asarray(inputs["db2"], f32)
    MsA = np.asarray(inputs["Ms_A"], f32)
    MfA = np.asarray(inputs["Mf_A"], f32)

    in_maps = []
    diag = np.arange(LOC)
    for c in range(ncores):
        r0, r1 = c * LOC, (c + 1) * LOC
        As_c = (-a_s) * Ls[r0:r1, :].T          # [N, LOC]
        As_c[r0 + diag, diag] += f32(1.0)
        Af_c = (-a_f) * Lf[r0:r1, :].T
        Af_c[r0 + diag, diag] += f32(1.0)
        m = dict(com)
        m.update({
            "As": pmajor(As_c).astype(NP_BF16),
            "Af": pmajor(Af_c).astype(NP_BF16),
            "x0lb": np.ascontiguousarray(x0[:, r0:r1]),
            "MsAT": b16((b_s * MsA[r0:r1]).T),
            "MfAT": b16((b_f * MfA[r0:r1]).T),
            "W2": b16(W2[:, r0:r1]),
            "b2r": b16(b2[r0:r1].reshape(1, LOC)),
            "D2": b16(D2[:, r0:r1]),
            "db2r": b16(db2[r0:r1].reshape(1, LOC)),
        })
        in_maps.append(m)
    return in_maps, (a_s, a_f, b_s, b_f)


def assemble_outputs(results, ncores=NCORES):
    f32 = np.float32
    x1 = np.concatenate([np.asarray(results[c]["x1_o"], f32)
                         for c in range(ncores)], axis=1)
    us = np.concatenate([np.asarray(results[c]["us_o"])
                         for c in range(ncores)], axis=0).T.astype(f32)
    uf = np.concatenate([np.asarray(results[c]["uf_o"])
                         for c in range(ncores)], axis=0).T.astype(f32)
    return (np.ascontiguousarray(x1), np.ascontiguousarray(us),
            np.ascontiguousarray(uf))


_PROGRAM_CACHE = {}


def kernel(**inputs):
    """Full-input / full-output entry point for the harness."""
    in_maps, _scal = make_in_maps(inputs)
    key = "full"
    if key not in _PROGRAM_CACHE:
        _PROGRAM_CACHE[key] = build_program()
    nc = _PROGRAM_CACHE[key]

    res = run_bass_kernel_spmd(nc, in_maps, core_ids=list(range(NCORES)))
    return assemble_outputs(res.results)


# revision 12
# speedup vs baseline: 4.6989x; 1.0770x over previous
"""Trainium2 Bass kernel for the CoupledTauModel (gnn_message_passing).

Strategy (v4 — flipped matmul orientation, bf16, SBUF pinning)
--------------------------------------------------------------
The Euler recurrence  us' = relu(us@(I-a_s*Ls).T + b_s*(uf@Ms_B)@Ms_A.T)
is computed with the *state* as the 128x8 stationary operand and the folded
matrix A = I - a*L streamed from HBM as the bf16 *moving* operand in 512-col
chunks, accumulating PSUM tiles of shape [B=8, 512].

Sharding: core c owns output columns [c*LOC, (c+1)*LOC) of A (column/output
-node sharding).  After each phase the new local state slice [8, LOC] is
relu'd on the DVE, transposed on the PE (8x [8,128] -> [128,8]) back into
node-major layout and AllGathered per state, so each collective hides under
the other phase's compute.

Queue discipline: A-slab streams on the Sync HWDGE queue; exchange DMAs
(AG staging + state reload) live alone on the Scalar HWDGE queue so their
semaphore waits block nothing; GpSimd triggers only collectives (SWDGE
descriptor generation is ~30us for the strided reload, so it must stay on
HWDGE).  Half the A chunks are pinned in SBUF and interleaved with streamed
chunks so the PE (which consumes A at ~614 GB/s) never starves on HBM
(~358 GB/s) and HAM stays un-throttled.

Both rank-R projections are computed in one PSUM group per step:
stationary [us_k | uf_k] (16 cols) x moving [MsB_k | MfB_k] (10 cols) ->
[16, 10], whose off-diagonal blocks are qf and qs.

Everything on the matmul path is bf16 (verified 8.3e-3 worst rel err vs the
fp32 reference, under the 2e-2 gate); PSUM accumulation stays fp32.
"""

import math
from contextlib import ExitStack

import numpy as np
import ml_dtypes

import concourse.bass as bass
import concourse.mybir as mybir
import concourse.tile as tile
from concourse import bacc
from concourse.bass_utils import run_bass_kernel_spmd

AF = mybir.ActivationFunctionType
FP32 = mybir.dt.float32
BF16 = mybir.dt.bfloat16
NP_BF16 = ml_dtypes.bfloat16

# Full-size problem config (hardcoded; the harness always uses this shape).
N_FULL = 8192
B_FULL = 8
H_FULL = 128
R_FULL = 5
NCORES = 8
NSTEPS_FULL = 10
DT = 0.1


def build_program(N=N_FULL, B=B_FULL, H=H_FULL, R=R_FULL, ncores=NCORES,
                  nsteps=NSTEPS_FULL, kchunk=4, slab_bufs=4, pin_chunks=8,
                  do_ag=True):
    """Build the SPMD Bass program (identical on every core; per-core data
    comes from each core's input map)."""
    LOC = N // ncores          # output columns owned per core
    KT = N // 128              # contraction k-tiles
    MT = LOC // 128            # 128-row tiles per local slice
    NJ = LOC // 512            # psum output chunks per phase
    NCH = KT // kchunk         # A chunks per phase
    KH = KT // 2               # half of the k-tiles (W1/D1 staging)
    assert N % (128 * ncores) == 0 and H == 128 and LOC % 512 == 0
    assert KT % kchunk == 0 and pin_chunks <= NCH and KT % 2 == 0

    nc = bacc.Bacc("TRN2", target_bir_lowering=False, debug=False,
                   num_devices=ncores)

    # ---- DRAM inputs (bf16 unless noted) ----
    As_d = nc.dram_tensor("As", [128, KT, LOC], BF16, kind="ExternalInput")
    Af_d = nc.dram_tensor("Af", [128, KT, LOC], BF16, kind="ExternalInput")
    x0T_d = nc.dram_tensor("x0T", [N, B], BF16, kind="ExternalInput")
    x0lb_d = nc.dram_tensor("x0lb", [B, LOC], FP32, kind="ExternalInput")
    QW = 32 + R               # [MsB | zeros | MfB] padded stationary width
    MB_d = nc.dram_tensor("MBc", [N, QW], BF16, kind="ExternalInput")
    MsAT_d = nc.dram_tensor("MsAT", [R, LOC], BF16, kind="ExternalInput")
    MfAT_d = nc.dram_tensor("MfAT", [R, LOC], BF16, kind="ExternalInput")
    W1_d = nc.dram_tensor("W1r", [128, KT, H], BF16, kind="ExternalInput")
    W2_d = nc.dram_tensor("W2", [H, LOC], BF16, kind="ExternalInput")
    D1_d = nc.dram_tensor("D1r", [128, KT, H], BF16, kind="ExternalInput")
    D2_d = nc.dram_tensor("D2", [H, LOC], BF16, kind="ExternalInput")
    b1r_d = nc.dram_tensor("b1r", [1, H], BF16, kind="ExternalInput")
    b2r_d = nc.dram_tensor("b2r", [1, LOC], BF16, kind="ExternalInput")
    db1r_d = nc.dram_tensor("db1r", [1, H], BF16, kind="ExternalInput")
    db2r_d = nc.dram_tensor("db2r", [1, LOC], BF16, kind="ExternalInput")
    ones_d = nc.dram_tensor("ones8", [1, B], BF16, kind="ExternalInput")
    eye_d = nc.dram_tensor("eye8", [B, B], BF16, kind="ExternalInput")

    x1_o = nc.dram_tensor("x1_o", [B, LOC], FP32, kind="ExternalOutput")
    us_o = nc.dram_tensor("us_o", [128, LOC // 128, B], BF16,
                          kind="ExternalOutput")
    uf_o = nc.dram_tensor("uf_o", [128, LOC // 128, B], BF16,
                          kind="ExternalOutput")

    RG = [list(range(ncores))]

    with ExitStack() as ctx:
        tc = ctx.enter_context(tile.TileContext(nc))
        const = ctx.enter_context(tc.tile_pool(name="const", bufs=1))
        statep = ctx.enter_context(tc.tile_pool(name="state", bufs=3))
        slabp = ctx.enter_context(tc.tile_pool(name="slab", bufs=slab_bufs))
        smallp = ctx.enter_context(tc.tile_pool(name="small", bufs=1))
        newp = ctx.enter_context(tc.tile_pool(name="new", bufs=2))
        psum = ctx.enter_context(tc.tile_pool(name="psum", bufs=1, space="PSUM"))
        dram = ctx.enter_context(tc.tile_pool(name="dram", bufs=2, space="DRAM"))

        # ---- resident constants (scalar HWDGE queue) ----
        x0T_sb = const.tile([128, KT, B], BF16, tag="x0T")
        nc.scalar.dma_start(x0T_sb[:], x0T_d[:].rearrange("(k p) b -> p k b", p=128))
        x0lb_sb = const.tile([B, LOC], FP32, tag="x0lb")
        nc.scalar.dma_start(x0lb_sb[:], x0lb_d[:])
        MB_sb = const.tile([128, KT, QW], BF16, tag="MBc")
        nc.scalar.dma_start(MB_sb[:], MB_d[:].rearrange("(k p) r -> p k r", p=128))
        MsAT_sb = const.tile([R, LOC], BF16, tag="MsAT")
        nc.scalar.dma_start(MsAT_sb[:], MsAT_d[:])
        MfAT_sb = const.tile([R, LOC], BF16, tag="MfAT")
        nc.scalar.dma_start(MfAT_sb[:], MfAT_d[:])
        W2_sb = const.tile([H, LOC], BF16, tag="W2")
        nc.scalar.dma_start(W2_sb[:], W2_d[:])
        D2_sb = const.tile([H, LOC], BF16, tag="D2")
        nc.scalar.dma_start(D2_sb[:], D2_d[:])
        b1r_sb = const.tile([1, H], BF16, tag="b1r")
        nc.scalar.dma_start(b1r_sb[:], b1r_d[:])
        b2r_sb = const.tile([1, LOC], BF16, tag="b2r")
        nc.scalar.dma_start(b2r_sb[:], b2r_d[:])
        db1r_sb = const.tile([1, H], BF16, tag="db1r")
        nc.scalar.dma_start(db1r_sb[:], db1r_d[:])
        db2r_sb = const.tile([1, LOC], BF16, tag="db2r")
        nc.scalar.dma_start(db2r_sb[:], db2r_d[:])
        ones_sb = const.tile([1, B], BF16, tag="ones")
        nc.scalar.dma_start(ones_sb[:], ones_d[:])
        eye_sb = const.tile([B, B], BF16, tag="eye")
        nc.scalar.dma_start(eye_sb[:], eye_d[:])

        # ---- pinned A chunks: loaded once, reused every step ----
        pinned = [{}, {}]
        if pin_chunks:
            pinp = ctx.enter_context(tc.tile_pool(name="pin", bufs=1))
            for ph, A_d in enumerate([As_d, Af_d]):
                for i in range(pin_chunks):
                    # pin every other chunk so pinned/streamed interleave
                    c = min(2 * i, NCH - 1)
                    pt_ = pinp.tile([128, kchunk, LOC], BF16,
                                    tag=f"pin{ph}_{i}", name=f"pin{ph}_{i}")
                    nc.sync.dma_start(
                        pt_[:], A_d[:, c * kchunk:(c + 1) * kchunk, :])
                    pinned[ph][c] = pt_

        def bT_to_nT(src_bT, nm):
            """[B, LOC] bf16 SBUF -> node-major [128, MT, B] bf16 tile via PE
            transposes."""
            news = newp.tile([128, MT, B], BF16, tag="news", name=f"news_{nm}")
            for m in range(MT):
                pt = psum.tile([128, B], BF16, tag="pt", bufs=2,
                               name=f"pt_{nm}_{m}")
                nc.tensor.transpose(pt[:], src_bT[:, m * 128:(m + 1) * 128],
                                    eye_sb[:])
                nc.vector.tensor_copy(news[:, m, :], pt[:])
            return news

        def exchange(news, nm, dst):
            """AllGather one state's local update into `dst`, a [128, KT, B]
            slice of the combined state tile.  The exchange stays in p-major
            layout: agin is a contiguous copy of the news tile and the
            reload has 128-byte runs (vs 16-byte in node-major), so the
            whole chain fits under one phase of PE work.  All DMAs on the
            scalar HWDGE queue."""
            agin = dram.tile([128, MT, B], BF16, tag=f"agin_{nm[-1]}",
                             name=f"agin_{nm}")
            nc.scalar.dma_start(agin[:], news[:])
            agout = dram.tile([ncores, 128, MT, B], BF16,
                              tag=f"agout_{nm[-1]}", name=f"agout_{nm}")
            if do_ag:
                nc.gpsimd.collective_compute(
                    "AllGather", mybir.AluOpType.bypass, replica_groups=RG,
                    ins=[agin[:]], outs=[agout[:]])
            else:
                nc.gpsimd.dma_start(agout[0], agin[:])
            for r in range(ncores):
                nc.scalar.dma_start(
                    dst[:, r * MT:(r + 1) * MT, :], agout[r])
            return agin

        # ---- gate MLP:  gate = sigmoid(relu(x0@W1+b1)@W2+b2) ----
        W1a = slabp.tile([128, KH, H], BF16, tag="slab", name="W1a")
        nc.scalar.dma_start(W1a[:], W1_d[:, 0:KH, :])
        W1b = slabp.tile([128, KH, H], BF16, tag="slab", name="W1b")
        nc.scalar.dma_start(W1b[:], W1_d[:, KH:KT, :])
        hp = psum.tile([B, H], FP32, tag="q")
        for k in range(KT):
            w = W1a if k < KH else W1b
            nc.tensor.matmul(hp[:], x0T_sb[:, k, :], w[:, k % KH, :],
                             start=(k == 0), stop=False)
        nc.tensor.matmul(hp[:], ones_sb[:], b1r_sb[:], start=False, stop=True)
        h_sb = smallp.tile([B, H], BF16, tag="h")
        nc.scalar.activation(h_sb[:], hp[:], AF.Relu)
        htp = psum.tile([128, B], BF16, tag="pt", bufs=2)
        nc.tensor.transpose(htp[:], h_sb[:], eye_sb[:])
        hT_sb = smallp.tile([H, B], BF16, tag="hT")
        nc.vector.tensor_copy(hT_sb[:], htp[:])

        us0_bT = smallp.tile([B, LOC], BF16, tag="new_bT", bufs=2)
        uf0_bT = smallp.tile([B, LOC], BF16, tag="new_bT", bufs=2)
        for j in range(NJ):
            sl = slice(j * 512, (j + 1) * 512)
            gp = psum.tile([B, 512], FP32, tag=f"out{j}", bufs=2,
                           name=f"gp_{j}")
            nc.tensor.matmul(gp[:], hT_sb[:], W2_sb[:, sl],
                             start=True, stop=False)
            nc.tensor.matmul(gp[:], ones_sb[:], b2r_sb[:, sl],
                             start=False, stop=True)
            g = smallp.tile([B, 512], FP32, tag="tmpg", name=f"g_{j}")
            nc.scalar.activation(g[:], gp[:], AF.Sigmoid)
            u = smallp.tile([B, 512], FP32, tag="tmpu", name=f"u_{j}")
            nc.vector.tensor_mul(u[:], g[:], x0lb_sb[:, sl])
            nc.vector.tensor_copy(us0_bT[:, sl], u[:])
            nc.vector.tensor_sub(uf0_bT[:, sl], x0lb_sb[:, sl], u[:])

        news0 = bT_to_nT(us0_bT, "s0")
        newf0 = bT_to_nT(uf0_bT, "f0")
        stb = statep.tile([128, KT, 2 * B], BF16, tag="stb", name="stb_0")
        exchange(news0, "us0", stb[:, :, 0:B])
        exchange(newf0, "uf0", stb[:, :, B:2 * B])

        # ---- Euler steps ----
        last_agin = [None, None]
        for t in range(nsteps):
            stb_next = statep.tile([128, KT, 2 * B], BF16, tag="stb",
                                   name=f"stb_{t + 1}")
            qT = [None, None]
            for ph in range(2):
                A_d = As_d if ph == 0 else Af_d
                stA = stb[:, :, ph * B:(ph + 1) * B]     # stationary state
                MAT = MsAT_sb if ph == 0 else MfAT_sb
                nm = f"{t}_{'s' if ph == 0 else 'f'}"

                outs = [psum.tile([B, 512], FP32, tag=f"out{j}", bufs=2,
                                  name=f"out{nm}_{j}") for j in range(NJ)]
                # pinned chunks are interleaved with streamed ones so the PE
                # has immediate work while each streamed slab arrives
                first = True
                for c in range(NCH):
                    if c in pinned[ph]:
                        slab = pinned[ph][c]
                    else:
                        slab = slabp.tile([128, kchunk, LOC], BF16, tag="slab",
                                          name=f"slab_{nm}_{c}")
                        nc.sync.dma_start(
                            slab[:], A_d[:, c * kchunk:(c + 1) * kchunk, :])
                    for kk in range(kchunk):
                        k = c * kchunk + kk
                        for j in range(NJ):
                            nc.tensor.matmul(
                                outs[j][:], stA[:, k, :],
                                slab[:, kk, j * 512:(j + 1) * 512],
                                start=first, stop=False)
                        first = False

                if ph == 0:
                    # both rank-R projections, pre-transposed, in one PSUM
                    # group: [MsB_k | 0 | MfB_k].T @ [us_k | uf_k] = [37, 16]
                    # qsT = block [0:R, 8:16]   ((uf@MsB).T)
                    # qfT = block [32:32+R, 0:8]  ((us@MfB).T)
                    # (MfB sits at column 32 so both PSUM reads are
                    # partition-32-aligned.)
                    qp = psum.tile([QW, 2 * B], FP32, tag="q",
                                   name=f"qp_{t}")
                    for k in range(KT):
                        nc.tensor.matmul(qp[:], MB_sb[:, k, :], stb[:, k, :],
                                         start=(k == 0), stop=(k == KT - 1))
                    qTs = smallp.tile([R, B], BF16, tag="qT0", bufs=2,
                                      name=f"qTs_{t}")
                    nc.vector.tensor_copy(qTs[:], qp[0:R, B:2 * B])
                    qTf = smallp.tile([R, B], BF16, tag="qT1", bufs=2,
                                      name=f"qTf_{t}")
                    nc.vector.tensor_copy(qTf[:], qp[32:32 + R, 0:B])
                    qT = [qTs, qTf]

                for j in range(NJ):
                    nc.tensor.matmul(outs[j][:], qT[ph][:],
                                     MAT[:, j * 512:(j + 1) * 512],
                                     start=False, stop=True)
                new_bT = smallp.tile([B, LOC], BF16, tag="new_bT", bufs=2,
                                     name=f"newbT_{nm}")
                for j in range(NJ):
                    nc.vector.tensor_scalar_max(
                        new_bT[:, j * 512:(j + 1) * 512], outs[j][:], 0.0)
                news = bT_to_nT(new_bT, nm)
                last_agin[ph] = exchange(
                    news, nm, stb_next[:, :, ph * B:(ph + 1) * B])
            stb = stb_next

        # final local state -> outputs (DRAM->DRAM copy out of the AG input)
        nc.scalar.dma_start(us_o[:], last_agin[0][:])
        nc.scalar.dma_start(uf_o[:], last_agin[1][:])

        # ---- decoder:  x1 = softplus(relu((us+uf)@D1+db1)@D2+db2) ----
        lat = statep.tile([128, KT, B], BF16, tag="lat", bufs=1)
        nc.vector.tensor_add(lat[:], stb[:, :, 0:B], stb[:, :, B:2 * B])
        D1a = slabp.tile([128, KH, H], BF16, tag="slab", name="D1a")
        nc.scalar.dma_start(D1a[:], D1_d[:, 0:KH, :])
        D1b = slabp.tile([128, KH, H], BF16, tag="slab", name="D1b")
        nc.scalar.dma_start(D1b[:], D1_d[:, KH:KT, :])
        hdp = psum.tile([B, H], FP32, tag="q")
        for k in range(KT):
            w = D1a if k < KH else D1b
            nc.tensor.matmul(hdp[:], lat[:, k, :], w[:, k % KH, :],
                             start=(k == 0), stop=False)
        nc.tensor.matmul(hdp[:], ones_sb[:], db1r_sb[:], start=False, stop=True)
        hd_sb = smallp.tile([B, H], BF16, tag="h")
        nc.scalar.activation(hd_sb[:], hdp[:], AF.Relu)
        hdtp = psum.tile([128, B], BF16, tag="pt", bufs=2)
        nc.tensor.transpose(hdtp[:], hd_sb[:], eye_sb[:])
        hdT_sb = smallp.tile([H, B], BF16, tag="hT")
        nc.vector.tensor_copy(hdT_sb[:], hdtp[:])

        for j in range(NJ):
            sl = slice(j * 512, (j + 1) * 512)
            xp = psum.tile([B, 512], FP32, tag=f"out{j}", bufs=2,
                           name=f"xp_{j}")
            nc.tensor.matmul(xp[:], hdT_sb[:], D2_sb[:, sl],
                             start=True, stop=False)
            nc.tensor.matmul(xp[:], ones_sb[:], db2r_sb[:, sl],
                             start=False, stop=True)
            # stable softplus(y) = relu(y) + ln(1+exp(-|y|))
            xa = smallp.tile([B, 512], FP32, tag="tmpg", name=f"xa_{j}")
            nc.scalar.activation(xa[:], xp[:], AF.Abs)
            nc.scalar.activation(xa[:], xa[:], AF.Exp, scale=-1.0)
            nc.scalar.activation(xa[:], xa[:], AF.Ln, bias=1.0)
            xr = smallp.tile([B, 512], FP32, tag="tmpu", name=f"xr_{j}")
            nc.scalar.activation(xr[:], xp[:], AF.Relu)
            x1c = smallp.tile([B, 512], FP32, tag="x1c", bufs=2,
                              name=f"x1c_{j}")
            nc.vector.tensor_add(x1c[:], xr[:], xa[:])
            nc.scalar.dma_start(x1_o[:, sl], x1c[:])

    nc.compile()
    return nc


def make_in_maps(inputs, N=N_FULL, B=B_FULL, H=H_FULL, R=R_FULL,
                 ncores=NCORES):
    """Host-side prep: fold scalars/identity into the streamed matrices,
    cast to bf16, reshape p-major, and shard across cores."""
    LOC = N // ncores
    KT = N // 128
    f32 = np.float32

    def softplus(x):
        return np.log1p(np.exp(np.float64(x)))

    def pmajor(a):
        # [N, C] -> [128, KT, C] with element (p, k, c) = a[k*128+p, c]
        return np.ascontiguousarray(
            a.reshape(KT, 128, -1).transpose(1, 0, 2))

    def b16(a):
        return np.ascontiguousarray(np.asarray(a).astype(NP_BF16))

    a_s = f32(DT * (softplus(inputs["raw_cs"]) + 1e-4))
    a_f = f32(DT * (softplus(inputs["raw_cf"]) + 1e-4))
    b_s = f32(DT * (softplus(inputs["raw_lambda_s"]) + 1e-4))
    b_f = f32(DT * (softplus(inputs["raw_lambda_f"]) + 1e-4))

    Ls = np.asarray(inputs["Ls"], f32)
    Lf = np.asarray(inputs["Lf"], f32)
    x0 = np.asarray(inputs["x0"], f32)

    MBc = np.zeros((N, 32 + R), f32)
    MBc[:, 0:R] = np.asarray(inputs["Ms_B"], f32)
    MBc[:, 32:32 + R] = np.asarray(inputs["Mf_B"], f32)

    com = {
        "x0T": b16(x0.T),
        "MBc": b16(MBc),
        "W1r": pmajor(np.asarray(inputs["W1"], f32)).astype(NP_BF16),
        "D1r": pmajor(np.asarray(inputs["D1"], f32)).astype(NP_BF16),
        "b1r": b16(np.asarray(inputs["b1"], f32).reshape(1, H)),
        "db1r": b16(np.asarray(inputs["db1"], f32).reshape(1, H)),
        "ones8": np.ones((1, B), NP_BF16),
        "eye8": np.eye(B, dtype=NP_BF16),
    }
    W2 = np.asarray(inputs["W2"], f32)
    D2 = np.asarray(inputs["D2"], f32)
    b2 = np.asarray(inputs["b2"], f32)
    db2 = np.asarray(inputs["db2"], f32)
    MsA = np.asarray(inputs["Ms_A"], f32)
    MfA = np.asarray(inputs["Mf_A"], f32)

    in_maps = []
    diag = np.arange(LOC)
    for c in range(ncores):
        r0, r1 = c * LOC, (c + 1) * LOC
        As_c = (-a_s) * Ls[r0:r1, :].T          # [N, LOC]
        As_c[r0 + diag, diag] += f32(1.0)
        Af_c = (-a_f) * Lf[r0:r1, :].T
        Af_c[r0 + diag, diag] += f32(1.0)
        m = dict(com)
        m.update({
            "As": pmajor(As_c).astype(NP_BF16),
            "Af": pmajor(Af_c).astype(NP_BF16),
            "x0lb": np.ascontiguousarray(x0[:, r0:r1]),
            "MsAT": b16((b_s * MsA[r0:r1]).T),
            "MfAT": b16((b_f * MfA[r0:r1]).T),
            "W2": b16(W2[:, r0:r1]),
            "b2r": b16(b2[r0:r1].reshape(1, LOC)),
            "D2": b16(D2[:, r0:r1]),
            "db2r": b16(db2[r0:r1].reshape(1, LOC)),
        })
        in_maps.append(m)
    return in_maps, (a_s, a_f, b_s, b_f)


def assemble_outputs(results, ncores=NCORES):
    f32 = np.float32
    x1 = np.concatenate([np.asarray(results[c]["x1_o"], f32)
                         for c in range(ncores)], axis=1)
    def unp(a):
        # [128, MT, B] p-major -> [LOC, B]
        a = np.asarray(a)
        return a.transpose(1, 0, 2).reshape(-1, a.shape[2])
    us = np.concatenate([unp(results[c]["us_o"])
                         for c in range(ncores)], axis=0).T.astype(f32)
    uf = np.concatenate([unp(results[c]["uf_o"])
                         for c in range(ncores)], axis=0).T.astype(f32)
    return (np.ascontiguousarray(x1), np.ascontiguousarray(us),
            np.ascontiguousarray(uf))


_PROGRAM_CACHE = {}


def kernel(**inputs):
    """Full-input / full-output entry point for the harness."""
    in_maps, _scal = make_in_maps(inputs)
    key = "full"
    if key not in _PROGRAM_CACHE:
        _PROGRAM_CACHE[key] = build_program()
    nc = _PROGRAM_CACHE[key]

    res = run_bass_kernel_spmd(nc, in_maps, core_ids=list(range(NCORES)))
    return assemble_outputs(res.results)
